# revision 10
# baseline (speedup 1.0000x reference)
"""Trainium2 Bass kernel for EquivariantThreeHopGINE (GNN message passing).

Strategy (8 NeuronCores, SPMD):
  - Nodes partitioned 12500/core (dst-sharding); edges assigned to the core
    owning their dst, sorted by dst, padded to a common tile count.
  - Atom embedding folded on host into a 124-wide multi-hot matmul
    (index preprocessing only; all float math stays on device).
  - Per GINE layer: gather h[src] from a replicated fp16 node table in DRAM
    (indirect DMA, 128 rows/call), messages m = relu(h_src + T[et]) via
    PE matmuls + ACT relu, scatter-add via one-hot matmuls into 512-node
    windows, then the W1/W2 MLP + LayerNorm + gated residual per window.
  - h slices are AllGathered (fp16) between layers to refresh the table.
  - Jumping-knowledge head: weight-stationary matmuls over DMA-transposed
    h0..h3 window slices.
"""
import os
import sys
import types
import numpy as np

for _p in ("/opt/trn_rl_repo", "/root/.axon_site/_ro/trn_rl_repo"):
    if os.path.isdir(_p) and _p not in sys.path:
        sys.path.insert(0, _p)

import concourse.bass as bass
import concourse.tile as tile
import concourse.mybir as mybir
from concourse import bass_utils
from concourse.masks import make_identity
import bass_rust

f32 = mybir.dt.float32
f16 = mybir.dt.float16
i32 = mybir.dt.int32
Alu = mybir.AluOpType
Act = mybir.ActivationFunctionType

N = 100000
E = 300000
HID = 256
NC_CORES = 8
SLICE = N // NC_CORES          # 12500
W = 512                        # dst-window width (nodes)
NW = (SLICE + W - 1) // W      # 25
NPAD = NW * W                  # 12800

ELEMENTS = [5, 6, 7, 8, 14, 15, 16]
ELEMENT_LUT = np.zeros(17, dtype=np.int64)
for _i, _z in enumerate(ELEMENTS):
    ELEMENT_LUT[_z] = _i
RING_VALS = np.array([0, 3, 4, 5, 6, 7, 8], dtype=np.int64)

# ---------------------------------------------------------------- compat shims


def _install_patches():
    """Split multi-sem-wait instructions: the public neuronxcc walrus codegen
    supports a single sync wait per instruction."""
    SC = bass_rust.ScopedClock

    def patched_drain(self, tick_clock, wait_clock):
        nc = self.nc
        drain_inst = nc.sync.drain()
        wait_clock.add_sem_waits(drain_inst.ins, SC({None: tick_clock.global_clock}))
        si = drain_inst.ins.sync_info
        waits = list(si.on_wait or [])
        if len(waits) > 1:
            si.on_wait = waits[:1]
            for w in waits[1:]:
                n = nc.sync.nop(nofuse=True)
                if n.ins.sync_info is None:
                    n.ins.sync_info = mybir.SyncInfo(on_wait=[w], on_update=[])
                else:
                    n.ins.sync_info.on_wait = [w]
        nc.all_engine_barrier()
        popped = nc._tile_sem_poison_stack.pop()
        assert popped is self._sem_poison
        nc.clear_and_free_semaphores(list(self.sems.allocated().values()))
        nc.all_engine_barrier()

    tile.TileContext._drain_and_barrier = patched_drain


_WAIT_UID = [0]


def _split_sync_waits(nc, maxw=1):
    for fn in nc.m.functions:
        for blk in fn.blocks:
            newlist = []
            for inst in blk.instructions:
                si = inst.sync_info
                if si is not None and si.on_wait and len(si.on_wait) > maxw:
                    waits = list(si.on_wait)
                    si.on_wait = waits[:maxw]
                    extra = waits[maxw:]
                    for i in range(0, len(extra), maxw):
                        _WAIT_UID[0] += 1
                        n = mybir.InstNoOp(name=f"waitnop_{_WAIT_UID[0]}", ins=[], outs=[])
                        n.engine = inst.engine
                        n.sync_info = mybir.SyncInfo(on_wait=list(extra[i:i + maxw]), on_update=[])
                        newlist.append(n)
                newlist.append(inst)
            blk.instructions[:] = newlist


def _install_ntff_hook():
    try:
        import antenv.axon_hooks  # noqa: F401
        return True
    except ImportError:
        pass
    try:
        if '/root/.axon_site' not in sys.path:
            sys.path.insert(0, '/root/.axon_site')
        from trn_agent_boot.trn_boot import _ntff_profile_via_ctypes
        hook = _ntff_profile_via_ctypes('/opt/axon/libaxon_pjrt.so')
        if hook is None:
            return False
        mod = types.ModuleType("antenv.axon_hooks")
        mod.get_axon_ntff_profile_hook = lambda: hook
        mod.set_axon_ntff_profile_hook = lambda h: None
        sys.modules["antenv.axon_hooks"] = mod
        import antenv
        antenv.axon_hooks = mod
        return True
    except Exception:
        return False


# ---------------------------------------------------------------- host folding

def _np(x):
    return np.asarray(x, dtype=np.float32)


def fold_params(p):
    """Fold the atom-embedding pipeline into A1[124,256], A2[48,256], c0[256]."""
    P, p_b = _np(p['disc_proj'][0]), _np(p['disc_proj'][1])      # [48,52],[48]
    L, l_b = _np(p['linear_0'][0]), _np(p['linear_0'][1])        # [256,64],[256]
    B, b_be = _np(p['bond_env_proj'][0]), _np(p['bond_env_proj'][1])  # [16,48],[16]
    R, r_b = _np(p['func_reduce'][0]), _np(p['func_reduce'][1])  # [4,36],[4]
    L48 = L[:, :48]   # [256,48]
    L16 = L[:, 48:]   # [256,16]
    G = L48 @ P       # [256,52]

    # table spec: (52-offset, dim, rows[n,dim])
    specs = []
    specs.append((0, 4, _np(p['element'])))      # 7
    specs.append((4, 4, _np(p['degree'])))       # 7
    specs.append((8, 4, _np(p['ring'])))         # 2
    specs.append((12, 4, _np(p['charge'])))      # 8
    specs.append((16, 4, _np(p['aromatic'])))    # 2
    specs.append((20, 4, _np(p['hybrid'])))      # 6
    specs.append((24, 4, _np(p['hydrogen'])))    # 5
    func = _np(p['func'])                        # [18,2,2]
    for j in range(18):
        rows = func[j] @ R[:, 2 * j:2 * j + 2].T   # [2,4] contribution to flags4
        specs.append((28, 4, rows))
    specs.append((32, 2, _np(p['h_don'])))       # 2
    specs.append((34, 2, _np(p['h_acc'])))       # 2
    specs.append((36, 4, _np(p['ringsize'])))    # 7
    specs.append((40, 4, _np(p['aroma_num'])))   # 5
    specs.append((44, 4, _np(p['fused_if'])))    # 8
    specs.append((48, 4, _np(p['het27'])))       # 27

    rows = []
    for off, dim, tab in specs:
        for k in range(tab.shape[0]):
            v52 = np.zeros(52, np.float32)
            v52[off:off + dim] = tab[k]
            rows.append(v52 @ G.T)               # [256]
    A1 = np.stack(rows, 0)                       # [124, 256]
    assert A1.shape[0] == 124

    const52 = np.zeros(52, np.float32)
    const52[28:32] = r_b
    c0 = const52 @ G.T + p_b @ L48.T + b_be @ L16.T + l_b   # [256]
    A1 = np.concatenate([A1, c0[None, :]], 0)                # [125, 256]
    A2 = B.T @ L16.T                                         # [48, 256]
    return A1, A2


def build_H124(xd):
    """Multi-hot index matrix from discrete atom columns (pure indexing)."""
    n = xd.shape[0]
    xi = xd.astype(np.int64)

    def clip(c, hi):
        return np.clip(xi[:, c], 0, hi)

    idxs = []
    z = xi[:, 0]
    z_safe = np.where((z >= 0) & (z <= 16), z, 0)
    idxs.append(np.clip(ELEMENT_LUT[z_safe], 0, 6))          # element 7
    idxs.append(clip(1, 6))                                   # degree 7
    idxs.append(np.clip(xi[:, 5] + 1, 0, 1))                  # ring 2
    idxs.append(clip(2, 7))                                   # charge 8
    idxs.append(clip(4, 1))                                   # aromatic 2
    idxs.append(clip(3, 5))                                   # hybrid 6
    idxs.append(clip(6, 4))                                   # hydrogen 5
    for j in range(18):
        idxs.append(clip(7 + j, 1))                           # flags 2 each
    idxs.append(clip(25, 1))                                  # h_don 2
    idxs.append(clip(26, 1))                                  # h_acc 2
    raw27 = xi[:, 27]
    match = raw27[:, None] == RING_VALS[None, :]
    mapped27 = np.where(match.any(1), match.argmax(1), 6)
    idxs.append(mapped27)                                     # ringsize 7
    idxs.append(clip(28, 4))                                  # aroma_num 5
    idxs.append(clip(29, 7))                                  # fused_if 8
    idxs.append(clip(30, 26))                                 # het27 27

    sizes = [7, 7, 2, 8, 2, 6, 5] + [2] * 18 + [2, 2, 7, 5, 8, 27]
    H = np.zeros((n, 124), np.float16)
    base = 0
    for sz, ix in zip(sizes, idxs):
        H[np.arange(n), base + ix] = 1.0
        base += sz
    assert base == 124
    return H


def prep_graph(edge_index, edge_type):
    """Per-core dst-sharded, dst-sorted edge arrays + shared tiling structure."""
    src = np.asarray(edge_index[0]).astype(np.int64)
    dst = np.asarray(edge_index[1]).astype(np.int64)
    et = np.asarray(edge_type).astype(np.int64)

    cores = []
    for k in range(NC_CORES):
        lo, hi = SLICE * k, SLICE * (k + 1)
        sel = (dst >= lo) & (dst < hi)
        d = dst[sel] - lo
        order = np.argsort(d, kind='stable')
        cores.append((src[sel][order], d[order], np.clip(et[sel][order], 0, 4)))

    emax = max(len(c[0]) for c in cores)
    ntiles = (emax + 127) // 128
    epad = ntiles * 128

    gsrcs, dstlocs, esels = [], [], []
    intervals = [[ntiles, 0] for _ in range(NW)]
    for k in range(NC_CORES):
        s, d, t = cores[k]
        ne = len(s)
        gs = np.zeros(epad, np.int32)
        gs[:ne] = s
        dl = np.full(epad, -100000.0, np.float32)
        dl[:ne] = d
        es = np.zeros((5, epad), np.float16)
        es[t, np.arange(ne)] = 1.0
        gsrcs.append(gs.reshape(ntiles, 128).T.copy())        # [128, ntiles]
        dstlocs.append(dl.reshape(ntiles, 128).T.copy())      # [128, ntiles]
        esels.append(es)
        wofe = d // W                                         # window of edge
        for w in range(NW):
            pos = np.nonzero(wofe == w)[0]
            if len(pos):
                intervals[w][0] = min(intervals[w][0], int(pos[0]) // 128)
                intervals[w][1] = max(intervals[w][1], int(pos[-1]) // 128 + 1)
    for w in range(NW):
        if intervals[w][0] >= intervals[w][1]:
            intervals[w] = [0, 0]
    return gsrcs, dstlocs, esels, ntiles, epad, [tuple(x) for x in intervals]


# ---------------------------------------------------------------- bass program

def build_program(ntiles, epad, intervals, one_minus_r, ln0_trivial):
    nc = bass.Bass("TRN2", target_bir_lowering=False, debug=False,
                   num_devices=NC_CORES)

    def din(name, shape, dt):
        return nc.dram_tensor(name, shape, dt, kind="ExternalInput").ap()

    HT = din("HT", [125, NPAD], f16)
    BT = din("BT", [48, NPAD], f16)
    gsrc = din("gsrc", [128, ntiles], i32)
    dstloc = din("dstloc", [128, ntiles], f32)
    esel = din("esel", [5, epad], f16)
    iota = din("iota", [128, W], f16)
    A1 = din("A1", [125, 256], f16)
    A2 = din("A2", [48, 256], f16)
    g0t = din("g0t", [128, 256], f16)
    b0t = din("b0t", [128, 256], f16)
    W1T = [din(f"W1T{i}", [128, 2, 256], f16) for i in range(3)]
    W2T = [din(f"W2T{i}", [128, 2, 256], f16) for i in range(3)]
    b1 = [din(f"b1_{i}", [128, 2], f32) for i in range(3)]
    b2 = [din(f"b2_{i}", [128, 2], f32) for i in range(3)]
    gmt = [din(f"gmt{i}", [128, 256], f16) for i in range(3)]   # r*gamma tiled
    btt = [din(f"btt{i}", [128, 256], f16) for i in range(3)]   # r*beta tiled
    Ttab = [din(f"Ttab{i}", [5, 256], f16) for i in range(3)]
    jkWT = din("jkWT", [128, 8, 256], f16)
    jkb = din("jkb", [128, 2], f32)
    outT = nc.dram_tensor("outT", [256, NPAD], f32, kind="ExternalOutput").ap()

    from contextlib import ExitStack
    with tile.TileContext(nc, num_cores=NC_CORES) as tc:
        with ExitStack() as ctx:
            ent = ctx.enter_context
            dram = ent(tc.tile_pool(name="dram", bufs=1, space="DRAM"))
            cst = ent(tc.tile_pool(name="consts", bufs=1))
            p_hsrc = ent(tc.tile_pool(name="p_hsrc", bufs=24))
            p_esel = ent(tc.tile_pool(name="p_esel", bufs=24))
            p_hb = ent(tc.tile_pool(name="p_hb", bufs=3))
            p_m = ent(tc.tile_pool(name="p_m", bufs=24))
            p_S = ent(tc.tile_pool(name="p_S", bufs=6))
            p_small = ent(tc.tile_pool(name="p_small", bufs=16))
            p_hwin = ent(tc.tile_pool(name="p_hwin", bufs=2))
            p_xw = ent(tc.tile_pool(name="p_xw", bufs=2))
            p_act = ent(tc.tile_pool(name="p_act", bufs=4))
            p_ln = ent(tc.tile_pool(name="p_ln", bufs=8))
            p_hn = ent(tc.tile_pool(name="p_hn", bufs=2))
            p_jk = ent(tc.tile_pool(name="p_jk", bufs=12))
            p_out = ent(tc.tile_pool(name="p_out", bufs=4))
            ps_agg = ent(tc.tile_pool(name="ps_agg", bufs=2, space="PSUM"))
            ps_mlp = ent(tc.tile_pool(name="ps_mlp", bufs=2, space="PSUM"))
            ps_h16 = ent(tc.tile_pool(name="ps_h16", bufs=1, space="PSUM"))
            ps_gt = ent(tc.tile_pool(name="ps_gt", bufs=1, space="PSUM"))
            ps_el = ent(tc.tile_pool(name="ps_el", bufs=2, space="PSUM"))

            # DRAM scratch
            hs = [dram.tile([NPAD, 256], f16, name=f"hs{i}") for i in range(4)]
            ag_in = dram.tile([SLICE, 256], f16, name="ag_in")
            tables = [dram.tile([N, 256], f16, addr_space="Shared", name=f"table{i}")
                      for i in range(3)]

            # load constants to SBUF
            def load(ap_in, shape, dt, name):
                t = cst.tile(shape, dt, name=name)
                nc.sync.dma_start(out=t[:], in_=ap_in)
                return t

            gsrc_s = load(gsrc, [128, ntiles], i32, "gsrc_s")
            dstloc_s = load(dstloc, [128, ntiles], f32, "dstloc_s")
            iota_s = load(iota, [128, W], f16, "iota_s")
            A1_s = load(A1, [125, 256], f16, "A1_s")
            A2_s = load(A2, [48, 256], f16, "A2_s")
            g0t_s = load(g0t, [128, 256], f16, "g0t_s")
            b0t_s = load(b0t, [128, 256], f16, "b0t_s")
            W1T_s = [load(W1T[i], [128, 2, 256], f16, f"W1T_s{i}") for i in range(3)]
            W2T_s = [load(W2T[i], [128, 2, 256], f16, f"W2T_s{i}") for i in range(3)]
            b1_s = [load(b1[i], [128, 2], f32, f"b1_s{i}") for i in range(3)]
            b2_s = [load(b2[i], [128, 2], f32, f"b2_s{i}") for i in range(3)]
            gmt_s = [load(gmt[i], [128, 256], f16, f"gmt_s{i}") for i in range(3)]
            btt_s = [load(btt[i], [128, 256], f16, f"btt_s{i}") for i in range(3)]
            Ttab_s = [load(Ttab[i], [5, 256], f16, f"Ttab_s{i}") for i in range(3)]
            jkWT_s = load(jkWT, [128, 8, 256], f16, "jkWT_s")
            jkb_s = load(jkb, [128, 2], f32, "jkb_s")

            ident = cst.tile([128, 128], f16, name="ident")
            make_identity(nc, ident[:])
            eps_s = cst.tile([128, 1], f32, name="eps_s")
            nc.vector.memset(eps_s[:], 1e-5)

            def write_ag_in(hn_t, w):
                base = W * w
                if base + W <= SLICE:
                    nc.sync.dma_start(
                        out=ag_in[base:base + W, :].rearrange("(c p) f -> p c f", p=128),
                        in_=hn_t[:])
                else:
                    rem = SLICE - base          # 212 for the last window
                    full = rem // 128
                    for j in range(full):
                        nc.sync.dma_start(out=ag_in[base + 128 * j:base + 128 * (j + 1), :],
                                          in_=hn_t[:, j, :])
                    part = rem - full * 128
                    if part:
                        nc.sync.dma_start(out=ag_in[base + full * 128:SLICE, :],
                                          in_=hn_t[:part, full, :])

            def layernorm_affine(gT_ps, gamt, bett, hwin_s, omr, hn_t, j):
                """Node-major LN of one [128,256] chunk + affine + residual.
                gT_ps: psum [128n, 256f]; hwin_s: [128,4,256] or None;
                writes hn_t[:, j, :]."""
                stats = p_ln.tile([128, 6], f32, name="stats", tag="stats")
                nc.vector.bn_stats(out=stats[:], in_=gT_ps)
                mv = p_ln.tile([128, 2], f32, name="mv", tag="mv")
                nc.vector.bn_aggr(out=mv[:], in_=stats[:])
                std = p_ln.tile([128, 1], f32, name="std", tag="std")
                nc.scalar.activation(out=std[:], in_=mv[:, 1:2], func=Act.Sqrt,
                                     bias=eps_s[:, :1], scale=1.0)
                rstd = p_ln.tile([128, 1], f32, name="rstd", tag="rstd")
                nc.vector.reciprocal(out=rstd[:], in_=std[:])
                if gamt is None and hwin_s is None:
                    nc.vector.tensor_scalar(out=hn_t[:, j, :], in0=gT_ps,
                                            scalar1=mv[:, 0:1], scalar2=rstd[:, 0:1],
                                            op0=Alu.subtract, op1=Alu.mult)
                    return
                u = p_ln.tile([128, 256], f16, name="u", tag="u")
                nc.vector.tensor_scalar(out=u[:], in0=gT_ps, scalar1=mv[:, 0:1],
                                        scalar2=rstd[:, 0:1],
                                        op0=Alu.subtract, op1=Alu.mult)
                v = p_ln.tile([128, 256], f16, name="v", tag="v")
                nc.vector.tensor_mul(out=v[:], in0=u[:], in1=gamt[:])
                if hwin_s is None:
                    nc.vector.tensor_add(out=hn_t[:, j, :], in0=v[:], in1=bett[:])
                else:
                    v2 = p_ln.tile([128, 256], f16, name="v2", tag="v2")
                    nc.vector.tensor_add(out=v2[:], in0=v[:], in1=bett[:])
                    hres = p_ln.tile([128, 256], f16, name="hres", tag="hres")
                    nc.scalar.mul(out=hres[:], in_=hwin_s[:, j, :], mul=float(omr))
                    nc.vector.tensor_add(out=hn_t[:, j, :], in0=v2[:], in1=hres[:])

            # ---------------- phase 0: atom embed -> h0
            for w in range(NW):
                ncol = slice(W * w, W * (w + 1))
                ht_t = p_hb.tile([125, W], f16, name="ht_t", tag="ht")
                nc.sync.dma_start(out=ht_t[:], in_=HT[:, ncol])
                bt_t = p_hb.tile([48, W], f16, name="bt_t", tag="bt")
                nc.sync.dma_start(out=bt_t[:], in_=BT[:, ncol])
                pre = [ps_mlp.tile([128, W], f32, name=f"pre{fc}", tag="mlp")
                       for fc in range(2)]
                for fc in range(2):
                    nc.tensor.matmul(out=pre[fc][:], lhsT=A1_s[:, 128 * fc:128 * (fc + 1)],
                                     rhs=ht_t[:], start=True, stop=False)
                    nc.tensor.matmul(out=pre[fc][:], lhsT=A2_s[:, 128 * fc:128 * (fc + 1)],
                                     rhs=bt_t[:], start=False, stop=True)
                pre_sb = p_xw.tile([128, 2, W], f16, name="pre_sb", tag="xw")
                for fc in range(2):
                    nc.scalar.activation(out=pre_sb[:, fc, :], in_=pre[fc][:],
                                         func=Act.Copy)
                gT = ps_gt.tile([128, 4, 256], f16, name="gT", tag="gt")
                for j in range(4):
                    for fc in range(2):
                        nc.tensor.transpose(
                            out=gT[:, j, 128 * fc:128 * (fc + 1)],
                            in_=pre_sb[:, fc, 128 * j:128 * (j + 1)],
                            identity=ident[:])
                hn = p_hn.tile([128, 4, 256], f16, name="hn", tag="hn")
                for j in range(4):
                    layernorm_affine(gT[:, j, :],
                                     None if ln0_trivial else g0t_s,
                                     None if ln0_trivial else b0t_s,
                                     None, 0.0, hn, j)
                nc.sync.dma_start(
                    out=hs[0][:].rearrange("(c p) f -> p c f", p=128)[:, 4 * w:4 * (w + 1), :],
                    in_=hn[:])
                write_ag_in(hn, w)

            def jk_window(w):
                outp = [ps_mlp.tile([128, W], f32, name=f"op{oc}", tag="mlp")
                        for oc in range(2)]
                for c in range(8):
                    i4, fc = c // 2, c % 2
                    hTc = p_jk.tile([128, W], f16, name=f"hTc{c}", tag="jk")
                    nc.sync.dma_start(
                        out=hTc[:],
                        in_=hs[i4][W * w:W * (w + 1), 128 * fc:128 * (fc + 1)],
                        transpose=True)
                    for oc in range(2):
                        nc.tensor.matmul(out=outp[oc][:],
                                         lhsT=jkWT_s[:, c, 128 * oc:128 * (oc + 1)],
                                         rhs=hTc[:], start=(c == 0), stop=(c == 7))
                for oc in range(2):
                    ob = p_out.tile([128, W], f32, name="ob", tag="ob")
                    nc.vector.tensor_scalar(out=ob[:], in0=outp[oc][:],
                                            scalar1=jkb_s[:, oc:oc + 1],
                                            scalar2=None, op0=Alu.add)
                    nc.sync.dma_start(out=outT[128 * oc:128 * (oc + 1), W * w:W * (w + 1)],
                                      in_=ob[:])

            # ---------------- 3 GINE layers
            for li in range(3):
                # allgather h_li into table
                table = tables[li]
                nc.gpsimd.collective_compute(
                    "AllGather", Alu.bypass,
                    replica_groups=[list(range(NC_CORES))],
                    ins=[ag_in[:]], outs=[table[:]])

                m_tiles = {}

                def make_m(t, li=li):
                    hsrc_t = p_hsrc.tile([128, 256], f16, name=f"hsrc{t}", tag="hsrc")
                    nc.gpsimd.indirect_dma_start(
                        out=hsrc_t[:], out_offset=None, in_=table[:],
                        in_offset=bass.IndirectOffsetOnAxis(ap=gsrc_s[:, t:t + 1], axis=0))
                    esel_t = p_esel.tile([5, 128], f16, name=f"esel{t}", tag="esel")
                    nc.sync.dma_start(out=esel_t[:], in_=esel[:, 128 * t:128 * (t + 1)])
                    el = ps_el.tile([128, 256], f32, name="el", tag="el")
                    nc.tensor.matmul(out=el[:], lhsT=esel_t[:],
                                     rhs=Ttab_s[li][:], start=True, stop=False)
                    nc.tensor.matmul(out=el[:], lhsT=ident[:], rhs=hsrc_t[:],
                                     start=False, stop=True)
                    m_t = p_m.tile([128, 256], f16, name=f"m{t}", tag="m")
                    nc.scalar.activation(out=m_t[:], in_=el[:], func=Act.Relu)
                    m_tiles[t] = m_t
                    return m_t

                for w in range(NW):
                    ncol = slice(W * w, W * (w + 1))
                    # load own h window (node-major) for residual + h+agg
                    hwin = p_hwin.tile([128, 4, 256], f16, name="hwin", tag="hwin")
                    nc.sync.dma_start(
                        out=hwin[:],
                        in_=hs[li][:].rearrange("(c p) f -> p c f", p=128)[:, 4 * w:4 * (w + 1), :])
                    # h window feature-major via DMA transpose
                    hT_sb = p_xw.tile([128, 2, W], f16, name="hT_sb", tag="hTsb")
                    for fc in range(2):
                        nc.sync.dma_start(
                            out=hT_sb[:, fc, :],
                            in_=hs[li][W * w:W * (w + 1), 128 * fc:128 * (fc + 1)],
                            transpose=True)

                    # scatter: one-hot matmuls over the window's tile interval
                    t_lo, t_hi = intervals[w]
                    agg = [ps_agg.tile([128, W], f32, name=f"agg{fc}", tag="agg")
                           for fc in range(2)]
                    first = True
                    for t in range(t_lo, t_hi):
                        m_t = m_tiles.get(t)
                        if m_t is None:
                            m_t = make_m(t)
                        sh = p_small.tile([128, 1], f32, name="sh", tag="sh")
                        nc.vector.tensor_scalar(out=sh[:], in0=dstloc_s[:, t:t + 1],
                                                scalar1=float(W * w), scalar2=None,
                                                op0=Alu.subtract)
                        S_t = p_S.tile([128, W], f16, name="S_t", tag="S")
                        nc.vector.tensor_scalar(out=S_t[:], in0=iota_s[:],
                                                scalar1=sh[:, 0:1], scalar2=None,
                                                op0=Alu.is_equal)
                        for fc in range(2):
                            nc.tensor.matmul(out=agg[fc][:],
                                             lhsT=m_t[:, 128 * fc:128 * (fc + 1)],
                                             rhs=S_t[:], start=first,
                                             stop=(t == t_hi - 1))
                        first = False
                    if first:  # empty interval (shouldn't happen)
                        for fc in range(2):
                            nc.vector.memset(agg[fc][:], 0.0)

                    # xw = h + agg  (feature-major fp16)
                    xw = p_xw.tile([128, 2, W], f16, name="xw", tag="xw")
                    for fc in range(2):
                        nc.vector.tensor_add(out=xw[:, fc, :], in0=hT_sb[:, fc, :],
                                             in1=agg[fc][:])
                    # W1 -> relu -> W2 -> relu
                    y1p = [ps_mlp.tile([128, W], f32, name=f"y1p{fc}", tag="mlp")
                           for fc in range(2)]
                    for fc in range(2):
                        for ki in range(2):
                            nc.tensor.matmul(out=y1p[fc][:],
                                             lhsT=W1T_s[li][:, ki, 128 * fc:128 * (fc + 1)],
                                             rhs=xw[:, ki, :],
                                             start=(ki == 0), stop=(ki == 1))
                    y1 = p_act.tile([128, W], f16, name="y1", tag="act")
                    y1b = p_act.tile([128, W], f16, name="y1b", tag="act")
                    ys = [y1, y1b]
                    for fc in range(2):
                        nc.scalar.activation(out=ys[fc][:], in_=y1p[fc][:], func=Act.Relu,
                                             bias=b1_s[li][:, fc:fc + 1],
                                             scale=1.0)
                    y2p = [ps_mlp.tile([128, W], f32, name=f"y2p{fc}", tag="mlp")
                           for fc in range(2)]
                    for fc in range(2):
                        for ki in range(2):
                            nc.tensor.matmul(out=y2p[fc][:],
                                             lhsT=W2T_s[li][:, ki, 128 * fc:128 * (fc + 1)],
                                             rhs=ys[ki][:],
                                             start=(ki == 0), stop=(ki == 1))
                    g1 = p_act.tile([128, W], f16, name="g1", tag="act")
                    g2 = p_act.tile([128, W], f16, name="g2", tag="act")
                    gs = [g1, g2]
                    for fc in range(2):
                        nc.scalar.activation(out=gs[fc][:], in_=y2p[fc][:], func=Act.Relu,
                                             bias=b2_s[li][:, fc:fc + 1],
                                             scale=1.0)
                    # transpose g back to node-major; LN + affine + residual
                    gT = ps_gt.tile([128, 4, 256], f16, name="gT2", tag="gt")
                    for j in range(4):
                        for fc in range(2):
                            nc.tensor.transpose(
                                out=gT[:, j, 128 * fc:128 * (fc + 1)],
                                in_=gs[fc][:, 128 * j:128 * (j + 1)],
                                identity=ident[:])
                    hn = p_hn.tile([128, 4, 256], f16, name="hn2", tag="hn")
                    for j in range(4):
                        layernorm_affine(gT[:, j, :], gmt_s[li], btt_s[li],
                                         hwin, one_minus_r[li], hn, j)
                    nc.sync.dma_start(
                        out=hs[li + 1][:].rearrange("(c p) f -> p c f", p=128)[:, 4 * w:4 * (w + 1), :],
                        in_=hn[:])
                    if li < 2:
                        write_ag_in(hn, w)
                    else:
                        jk_window(w)

            pass

    _split_sync_waits(nc)
    return nc


# ---------------------------------------------------------------- entry point

_CACHE = {}


def kernel(atom_inputs, edge_index, edge_type, params):
    _install_patches()
    _install_ntff_hook()

    x = np.asarray(atom_inputs, dtype=np.float32)
    A1, A2 = fold_params(params)
    H = build_H124(x[:, :31])                     # [N,124] fp16
    H = np.concatenate([H, np.ones((H.shape[0], 1), np.float16)], 1)  # +const col
    benv = x[:, 31:].astype(np.float16)           # [N,48]
    gsrcs, dstlocs, esels, ntiles, epad, intervals = prep_graph(edge_index, edge_type)

    p = params
    one_minus_r = [1.0 - float(np.asarray(p[f'res{i}'])) for i in (1, 2, 3)]
    r_ = [float(np.asarray(p[f'res{i}'])) for i in (1, 2, 3)]

    common = {
        "iota": np.tile(np.arange(W, dtype=np.float16), (128, 1)),
        "A1": A1.astype(np.float16),
        "A2": A2.astype(np.float16),
        "g0t": np.tile(_np(p['ln_in'][0]).astype(np.float16), (128, 1)),
        "b0t": np.tile(_np(p['ln_in'][1]).astype(np.float16), (128, 1)),
        "jkWT": _np(p['jk'][0]).T.astype(np.float16).reshape(8, 128, 256).transpose(1, 0, 2).copy(),
        "jkb": _np(p['jk'][1]).astype(np.float32).reshape(2, 128).T.copy(),
    }
    for i0, i in enumerate((1, 2, 3)):
        common[f"W1T{i0}"] = _np(p[f'gine{i}_W1'][0]).T.astype(np.float16).reshape(2, 128, 256).transpose(1, 0, 2).copy()
        common[f"W2T{i0}"] = _np(p[f'gine{i}_W2'][0]).T.astype(np.float16).reshape(2, 128, 256).transpose(1, 0, 2).copy()
        common[f"b1_{i0}"] = _np(p[f'gine{i}_W1'][1]).astype(np.float32).reshape(2, 128).T.copy()
        common[f"b2_{i0}"] = _np(p[f'gine{i}_W2'][1]).astype(np.float32).reshape(2, 128).T.copy()
        common[f"gmt{i0}"] = np.tile((r_[i0] * _np(p[f'ln{i}'][0])).astype(np.float16), (128, 1))
        common[f"btt{i0}"] = np.tile((r_[i0] * _np(p[f'ln{i}'][1])).astype(np.float16), (128, 1))
        common[f"Ttab{i0}"] = (_np(p['bond_emb']) @ _np(p[f'gine{i}_lin'][0]).T
                               + _np(p[f'gine{i}_lin'][1])).astype(np.float16)

    in_maps = []
    for k in range(NC_CORES):
        lo, hi = SLICE * k, SLICE * (k + 1)
        HTk = np.zeros((125, NPAD), np.float16)
        HTk[:, :SLICE] = H[lo:hi].T
        BTk = np.zeros((48, NPAD), np.float16)
        BTk[:, :SLICE] = benv[lo:hi].T
        m = dict(common)
        m.update({"HT": HTk, "BT": BTk, "gsrc": gsrcs[k],
                  "dstloc": dstlocs[k], "esel": esels[k]})
        in_maps.append(m)

    ln0_trivial = bool(np.all(_np(p['ln_in'][0]) == 1.0) and np.all(_np(p['ln_in'][1]) == 0.0))
    key = (ntiles, epad, tuple(intervals), tuple(one_minus_r), ln0_trivial)
    nc = _CACHE.get(key)
    if nc is None:
        nc = build_program(ntiles, epad, intervals, one_minus_r, ln0_trivial)
        _CACHE[key] = nc

    trace = bool(int(os.environ.get("GINE_TRACE", "0")))
    res = bass_utils.run_bass_kernel_spmd(nc, in_maps,
                                          core_ids=list(range(NC_CORES)),
                                          trace=trace)
    kernel.last_exec_time_ns = res.exec_time_ns
    out = np.concatenate(
        [res.results[k]["outT"].T[:SLICE] for k in range(NC_CORES)], axis=0)
    return out.astype(np.float32)


kernel.last_exec_time_ns = None


# revision 14
# speedup vs baseline: 1.1830x; 1.1830x over previous
"""Trainium2 Bass kernel for EquivariantThreeHopGINE (GNN message passing).

Strategy (8 NeuronCores, SPMD):
  - Nodes partitioned 12500/core (dst-sharding); edges assigned to the core
    owning their dst, sorted by dst, padded to a common tile count.
  - Atom embedding folded on host into a 124-wide multi-hot matmul
    (index preprocessing only; all float math stays on device).
  - Per GINE layer: gather h[src] from a replicated fp16 node table in DRAM
    (indirect DMA, 128 rows/call), messages m = relu(h_src + T[et]) via
    PE matmuls + ACT relu, scatter-add via one-hot matmuls into 512-node
    windows, then the W1/W2 MLP + LayerNorm + gated residual per window.
  - h slices are AllGathered (fp16) between layers to refresh the table.
  - Jumping-knowledge head: weight-stationary matmuls over DMA-transposed
    h0..h3 window slices.
"""
import os
import sys
import types
import numpy as np

for _p in ("/opt/trn_rl_repo", "/root/.axon_site/_ro/trn_rl_repo"):
    if os.path.isdir(_p) and _p not in sys.path:
        sys.path.insert(0, _p)

import concourse.bass as bass
import concourse.tile as tile
import concourse.mybir as mybir
from concourse import bass_utils
from concourse.masks import make_identity
import bass_rust

f32 = mybir.dt.float32
f16 = mybir.dt.float16
i32 = mybir.dt.int32
Alu = mybir.AluOpType
Act = mybir.ActivationFunctionType

N = 100000
E = 300000
HID = 256
NC_CORES = 8
SLICE = N // NC_CORES          # 12500
W = 512                        # dst-window width (nodes)
NW = (SLICE + W - 1) // W      # 25
NPAD = NW * W                  # 12800
NSW = NPAD // 128              # 100 scatter sub-windows

ELEMENTS = [5, 6, 7, 8, 14, 15, 16]
ELEMENT_LUT = np.zeros(17, dtype=np.int64)
for _i, _z in enumerate(ELEMENTS):
    ELEMENT_LUT[_z] = _i
RING_VALS = np.array([0, 3, 4, 5, 6, 7, 8], dtype=np.int64)

# ---------------------------------------------------------------- compat shims


def _install_patches():
    """Split multi-sem-wait instructions: the public neuronxcc walrus codegen
    supports a single sync wait per instruction."""
    SC = bass_rust.ScopedClock

    def patched_drain(self, tick_clock, wait_clock):
        nc = self.nc
        drain_inst = nc.sync.drain()
        wait_clock.add_sem_waits(drain_inst.ins, SC({None: tick_clock.global_clock}))
        si = drain_inst.ins.sync_info
        waits = list(si.on_wait or [])
        if len(waits) > 1:
            si.on_wait = waits[:1]
            for w in waits[1:]:
                n = nc.sync.nop(nofuse=True)
                if n.ins.sync_info is None:
                    n.ins.sync_info = mybir.SyncInfo(on_wait=[w], on_update=[])
                else:
                    n.ins.sync_info.on_wait = [w]
        nc.all_engine_barrier()
        popped = nc._tile_sem_poison_stack.pop()
        assert popped is self._sem_poison
        nc.clear_and_free_semaphores(list(self.sems.allocated().values()))
        nc.all_engine_barrier()

    tile.TileContext._drain_and_barrier = patched_drain


_WAIT_UID = [0]


def _split_sync_waits(nc, maxw=1):
    for fn in nc.m.functions:
        for blk in fn.blocks:
            newlist = []
            for inst in blk.instructions:
                si = inst.sync_info
                if si is not None and si.on_wait and len(si.on_wait) > maxw:
                    waits = list(si.on_wait)
                    si.on_wait = waits[:maxw]
                    extra = waits[maxw:]
                    for i in range(0, len(extra), maxw):
                        _WAIT_UID[0] += 1
                        n = mybir.InstNoOp(name=f"waitnop_{_WAIT_UID[0]}", ins=[], outs=[])
                        n.engine = inst.engine
                        n.sync_info = mybir.SyncInfo(on_wait=list(extra[i:i + maxw]), on_update=[])
                        newlist.append(n)
                newlist.append(inst)
            blk.instructions[:] = newlist


def _install_ntff_hook():
    try:
        import antenv.axon_hooks  # noqa: F401
        return True
    except ImportError:
        pass
    try:
        if '/root/.axon_site' not in sys.path:
            sys.path.insert(0, '/root/.axon_site')
        from trn_agent_boot.trn_boot import _ntff_profile_via_ctypes
        hook = _ntff_profile_via_ctypes('/opt/axon/libaxon_pjrt.so')
        if hook is None:
            return False
        mod = types.ModuleType("antenv.axon_hooks")
        mod.get_axon_ntff_profile_hook = lambda: hook
        mod.set_axon_ntff_profile_hook = lambda h: None
        sys.modules["antenv.axon_hooks"] = mod
        import antenv
        antenv.axon_hooks = mod
        return True
    except Exception:
        return False


# ---------------------------------------------------------------- host folding

def _np(x):
    return np.asarray(x, dtype=np.float32)


def fold_params(p):
    """Fold the atom-embedding pipeline into A1[124,256], A2[48,256], c0[256]."""
    P, p_b = _np(p['disc_proj'][0]), _np(p['disc_proj'][1])      # [48,52],[48]
    L, l_b = _np(p['linear_0'][0]), _np(p['linear_0'][1])        # [256,64],[256]
    B, b_be = _np(p['bond_env_proj'][0]), _np(p['bond_env_proj'][1])  # [16,48],[16]
    R, r_b = _np(p['func_reduce'][0]), _np(p['func_reduce'][1])  # [4,36],[4]
    L48 = L[:, :48]   # [256,48]
    L16 = L[:, 48:]   # [256,16]
    G = L48 @ P       # [256,52]

    # table spec: (52-offset, dim, rows[n,dim])
    specs = []
    specs.append((0, 4, _np(p['element'])))      # 7
    specs.append((4, 4, _np(p['degree'])))       # 7
    specs.append((8, 4, _np(p['ring'])))         # 2
    specs.append((12, 4, _np(p['charge'])))      # 8
    specs.append((16, 4, _np(p['aromatic'])))    # 2
    specs.append((20, 4, _np(p['hybrid'])))      # 6
    specs.append((24, 4, _np(p['hydrogen'])))    # 5
    func = _np(p['func'])                        # [18,2,2]
    for j in range(18):
        rows = func[j] @ R[:, 2 * j:2 * j + 2].T   # [2,4] contribution to flags4
        specs.append((28, 4, rows))
    specs.append((32, 2, _np(p['h_don'])))       # 2
    specs.append((34, 2, _np(p['h_acc'])))       # 2
    specs.append((36, 4, _np(p['ringsize'])))    # 7
    specs.append((40, 4, _np(p['aroma_num'])))   # 5
    specs.append((44, 4, _np(p['fused_if'])))    # 8
    specs.append((48, 4, _np(p['het27'])))       # 27

    rows = []
    for off, dim, tab in specs:
        for k in range(tab.shape[0]):
            v52 = np.zeros(52, np.float32)
            v52[off:off + dim] = tab[k]
            rows.append(v52 @ G.T)               # [256]
    A1 = np.stack(rows, 0)                       # [124, 256]
    assert A1.shape[0] == 124

    const52 = np.zeros(52, np.float32)
    const52[28:32] = r_b
    c0 = const52 @ G.T + p_b @ L48.T + b_be @ L16.T + l_b   # [256]
    A1 = np.concatenate([A1, c0[None, :]], 0)                # [125, 256]
    A2 = B.T @ L16.T                                         # [48, 256]
    return A1, A2


def build_H124(xd):
    """Multi-hot index matrix from discrete atom columns (pure indexing)."""
    n = xd.shape[0]
    xi = xd.astype(np.int64)

    def clip(c, hi):
        return np.clip(xi[:, c], 0, hi)

    idxs = []
    z = xi[:, 0]
    z_safe = np.where((z >= 0) & (z <= 16), z, 0)
    idxs.append(np.clip(ELEMENT_LUT[z_safe], 0, 6))          # element 7
    idxs.append(clip(1, 6))                                   # degree 7
    idxs.append(np.clip(xi[:, 5] + 1, 0, 1))                  # ring 2
    idxs.append(clip(2, 7))                                   # charge 8
    idxs.append(clip(4, 1))                                   # aromatic 2
    idxs.append(clip(3, 5))                                   # hybrid 6
    idxs.append(clip(6, 4))                                   # hydrogen 5
    for j in range(18):
        idxs.append(clip(7 + j, 1))                           # flags 2 each
    idxs.append(clip(25, 1))                                  # h_don 2
    idxs.append(clip(26, 1))                                  # h_acc 2
    raw27 = xi[:, 27]
    match = raw27[:, None] == RING_VALS[None, :]
    mapped27 = np.where(match.any(1), match.argmax(1), 6)
    idxs.append(mapped27)                                     # ringsize 7
    idxs.append(clip(28, 4))                                  # aroma_num 5
    idxs.append(clip(29, 7))                                  # fused_if 8
    idxs.append(clip(30, 26))                                 # het27 27

    sizes = [7, 7, 2, 8, 2, 6, 5] + [2] * 18 + [2, 2, 7, 5, 8, 27]
    H = np.zeros((n, 124), np.float16)
    base = 0
    for sz, ix in zip(sizes, idxs):
        H[np.arange(n), base + ix] = 1.0
        base += sz
    assert base == 124
    return H


def prep_graph(edge_index, edge_type):
    """Per-core dst-sharded, dst-sorted edge arrays + shared tiling structure."""
    src = np.asarray(edge_index[0]).astype(np.int64)
    dst = np.asarray(edge_index[1]).astype(np.int64)
    et = np.asarray(edge_type).astype(np.int64)

    cores = []
    for k in range(NC_CORES):
        lo, hi = SLICE * k, SLICE * (k + 1)
        sel = (dst >= lo) & (dst < hi)
        d = dst[sel] - lo
        order = np.argsort(d, kind='stable')
        cores.append((src[sel][order], d[order], np.clip(et[sel][order], 0, 4)))

    emax = max(len(c[0]) for c in cores)
    ntiles = (emax + 127) // 128
    epad = ntiles * 128

    gsrcs, dstlocs, esels = [], [], []
    intervals = [[ntiles, 0] for _ in range(NSW)]
    for k in range(NC_CORES):
        s, d, t = cores[k]
        ne = len(s)
        gs = np.zeros(epad, np.int32)
        gs[:ne] = s
        dl = np.full(epad, -100000.0, np.float32)
        dl[:ne] = d
        es = np.zeros((5, epad), np.float16)
        es[t, np.arange(ne)] = 1.0
        gsrcs.append(gs.reshape(ntiles, 128).T.copy())        # [128, ntiles]
        dstlocs.append(dl.reshape(ntiles, 128).T.copy())      # [128, ntiles]
        esels.append(es)
        wofe = d // 128                                       # sub-window of edge
    for k in range(NC_CORES):
        s, d, t = cores[k]
        wofe = d // 128
        for w in range(NSW):
            pos = np.nonzero(wofe == w)[0]
            if len(pos):
                intervals[w][0] = min(intervals[w][0], int(pos[0]) // 128)
                intervals[w][1] = max(intervals[w][1], int(pos[-1]) // 128 + 1)
    for w in range(NSW):
        if intervals[w][0] >= intervals[w][1]:
            intervals[w] = [0, 0]
    return gsrcs, dstlocs, esels, ntiles, epad, [tuple(x) for x in intervals]


# ---------------------------------------------------------------- bass program

def build_program(ntiles, epad, intervals, one_minus_r, ln0_trivial):
    nc = bass.Bass("TRN2", target_bir_lowering=False, debug=False,
                   num_devices=NC_CORES)

    def din(name, shape, dt):
        return nc.dram_tensor(name, shape, dt, kind="ExternalInput").ap()

    HT = din("HT", [125, NPAD], f16)
    BT = din("BT", [48, NPAD], f16)
    gsrc = din("gsrc", [128, ntiles], i32)
    dstloc = din("dstloc", [128, ntiles], f32)
    esel = din("esel", [5, epad], f16)
    iota = din("iota", [128, NPAD], f32)
    A1 = din("A1", [125, 256], f16)
    A2 = din("A2", [48, 256], f16)
    g0t = din("g0t", [128, 256], f16)
    b0t = din("b0t", [128, 256], f16)
    W1T = [din(f"W1T{i}", [128, 2, 256], f16) for i in range(3)]
    W2T = [din(f"W2T{i}", [128, 2, 256], f16) for i in range(3)]
    b1 = [din(f"b1_{i}", [128, 2], f32) for i in range(3)]
    b2 = [din(f"b2_{i}", [128, 2], f32) for i in range(3)]
    gmt = [din(f"gmt{i}", [128, 256], f16) for i in range(3)]
    btt = [din(f"btt{i}", [128, 256], f16) for i in range(3)]
    Ttab = [din(f"Ttab{i}", [5, 256], f16) for i in range(3)]
    jkWT = din("jkWT", [128, 8, 256], f16)
    jkb = din("jkb", [128, 2], f32)
    outT = nc.dram_tensor("outT", [256, NPAD], f32, kind="ExternalOutput").ap()

    from contextlib import ExitStack
    with tile.TileContext(nc, num_cores=NC_CORES) as tc:
        with ExitStack() as ctx:
            ent = ctx.enter_context
            dram = ent(tc.tile_pool(name="dram", bufs=1, space="DRAM"))
            cst = ent(tc.tile_pool(name="consts", bufs=1))
            p_hsrc = ent(tc.tile_pool(name="p_hsrc", bufs=24))
            p_esel = ent(tc.tile_pool(name="p_esel", bufs=24))
            p_hb = ent(tc.tile_pool(name="p_hb", bufs=3))
            p_m = ent(tc.tile_pool(name="p_m", bufs=24))
            p_S = ent(tc.tile_pool(name="p_S", bufs=8))
            p_hwin = ent(tc.tile_pool(name="p_hwin", bufs=3))
            p_xw = ent(tc.tile_pool(name="p_xw", bufs=2))
            p_xwn = ent(tc.tile_pool(name="p_xwn", bufs=2))
            p_act = ent(tc.tile_pool(name="p_act", bufs=4))
            p_ln = ent(tc.tile_pool(name="p_ln", bufs=8))
            p_hn = ent(tc.tile_pool(name="p_hn", bufs=3))
            p_jk = ent(tc.tile_pool(name="p_jk", bufs=8))
            p_out = ent(tc.tile_pool(name="p_out", bufs=10))
            ps_agg = ent(tc.tile_pool(name="ps_agg", bufs=2, space="PSUM"))
            ps_mlp = ent(tc.tile_pool(name="ps_mlp", bufs=2, space="PSUM"))
            ps_t16 = ent(tc.tile_pool(name="ps_t16", bufs=2, space="PSUM"))
            ps_el = ent(tc.tile_pool(name="ps_el", bufs=2, space="PSUM"))

            hs = [dram.tile([NPAD, 256], f16, name=f"hs{i}") for i in range(4)]
            ag_in = dram.tile([SLICE, 256], f16, name="ag_in")
            tables = [dram.tile([N, 256], f16, addr_space="Shared", name=f"table{i}")
                      for i in range(3)]

            def load(ap_in, shape, dt, name):
                t = cst.tile(shape, dt, name=name)
                nc.sync.dma_start(out=t[:], in_=ap_in)
                return t

            gsrc_s = load(gsrc, [128, ntiles], i32, "gsrc_s")
            dstloc_s = load(dstloc, [128, ntiles], f32, "dstloc_s")
            iota_s = load(iota, [128, NPAD], f32, "iota_s")
            A1_s = load(A1, [125, 256], f16, "A1_s")
            A2_s = load(A2, [48, 256], f16, "A2_s")
            g0t_s = load(g0t, [128, 256], f16, "g0t_s")
            b0t_s = load(b0t, [128, 256], f16, "b0t_s")
            W1T_s = [load(W1T[i], [128, 2, 256], f16, f"W1T_s{i}") for i in range(3)]
            W2T_s = [load(W2T[i], [128, 2, 256], f16, f"W2T_s{i}") for i in range(3)]
            b1_s = [load(b1[i], [128, 2], f32, f"b1_s{i}") for i in range(3)]
            b2_s = [load(b2[i], [128, 2], f32, f"b2_s{i}") for i in range(3)]
            gmt_s = [load(gmt[i], [128, 256], f16, f"gmt_s{i}") for i in range(3)]
            btt_s = [load(btt[i], [128, 256], f16, f"btt_s{i}") for i in range(3)]
            Ttab_s = [load(Ttab[i], [5, 256], f16, f"Ttab_s{i}") for i in range(3)]
            jkWT_s = load(jkWT, [128, 8, 256], f16, "jkWT_s")
            jkb_s = load(jkb, [128, 2], f32, "jkb_s")

            ident = cst.tile([128, 128], f16, name="ident")
            make_identity(nc, ident[:])
            eps_s = cst.tile([128, 1], f32, name="eps_s")
            nc.vector.memset(eps_s[:], 1e-5)

            def write_ag_in(hn_t, w):
                base = W * w
                if base + W <= SLICE:
                    nc.sync.dma_start(
                        out=ag_in[base:base + W, :].rearrange("(c p) f -> p c f", p=128),
                        in_=hn_t[:])
                else:
                    rem = SLICE - base
                    full = rem // 128
                    for j in range(full):
                        nc.sync.dma_start(out=ag_in[base + 128 * j:base + 128 * (j + 1), :],
                                          in_=hn_t[:, j, :])
                    part = rem - full * 128
                    if part:
                        nc.sync.dma_start(out=ag_in[base + full * 128:SLICE, :],
                                          in_=hn_t[:part, full, :])

            def layernorm_affine(gT_ps, gamt, bett, hwin_s, omr, hn_t, j):
                stats = p_ln.tile([128, 6], f32, name="stats", tag="stats")
                nc.vector.bn_stats(out=stats[:], in_=gT_ps)
                mv = p_ln.tile([128, 2], f32, name="mv", tag="mv")
                nc.vector.bn_aggr(out=mv[:], in_=stats[:])
                std = p_ln.tile([128, 1], f32, name="std", tag="std")
                nc.scalar.activation(out=std[:], in_=mv[:, 1:2], func=Act.Sqrt,
                                     bias=eps_s[:, :1], scale=1.0)
                rstd = p_ln.tile([128, 1], f32, name="rstd", tag="rstd")
                nc.vector.reciprocal(out=rstd[:], in_=std[:])
                if gamt is None and hwin_s is None:
                    nc.vector.tensor_scalar(out=hn_t[:, j, :], in0=gT_ps,
                                            scalar1=mv[:, 0:1], scalar2=rstd[:, 0:1],
                                            op0=Alu.subtract, op1=Alu.mult)
                    return
                u = p_ln.tile([128, 256], f16, name="u", tag="u")
                nc.vector.tensor_scalar(out=u[:], in0=gT_ps, scalar1=mv[:, 0:1],
                                        scalar2=rstd[:, 0:1],
                                        op0=Alu.subtract, op1=Alu.mult)
                v = p_ln.tile([128, 256], f16, name="v", tag="v")
                nc.vector.tensor_mul(out=v[:], in0=u[:], in1=gamt[:])
                if hwin_s is None:
                    nc.vector.tensor_add(out=hn_t[:, j, :], in0=v[:], in1=bett[:])
                else:
                    v2 = p_ln.tile([128, 256], f16, name="v2", tag="v2")
                    nc.vector.tensor_add(out=v2[:], in0=v[:], in1=bett[:])
                    hres = p_ln.tile([128, 256], f16, name="hres", tag="hres")
                    nc.scalar.mul(out=hres[:], in_=hwin_s[:, j, :], mul=float(omr))
                    nc.vector.tensor_add(out=hn_t[:, j, :], in0=v2[:], in1=hres[:])

            # ---------------- phase 0: atom embed -> h0
            for w in range(NW):
                ncol = slice(W * w, W * (w + 1))
                ht_t = p_hb.tile([125, W], f16, name="ht_t", tag="ht")
                nc.sync.dma_start(out=ht_t[:], in_=HT[:, ncol])
                bt_t = p_hb.tile([48, W], f16, name="bt_t", tag="bt")
                nc.sync.dma_start(out=bt_t[:], in_=BT[:, ncol])
                pre = [ps_mlp.tile([128, W], f32, name=f"pre{fc}", tag="mlp")
                       for fc in range(2)]
                for fc in range(2):
                    nc.tensor.matmul(out=pre[fc][:], lhsT=A1_s[:, 128 * fc:128 * (fc + 1)],
                                     rhs=ht_t[:], start=True, stop=False)
                    nc.tensor.matmul(out=pre[fc][:], lhsT=A2_s[:, 128 * fc:128 * (fc + 1)],
                                     rhs=bt_t[:], start=False, stop=True)
                pre_sb = p_xw.tile([128, 2, W], f16, name="pre_sb", tag="xw")
                for fc in range(2):
                    nc.scalar.activation(out=pre_sb[:, fc, :], in_=pre[fc][:],
                                         func=Act.Copy)
                gT = ps_t16.tile([128, 4, 256], f16, name="gT", tag="t16")
                for j in range(4):
                    for fc in range(2):
                        nc.tensor.transpose(
                            out=gT[:, j, 128 * fc:128 * (fc + 1)],
                            in_=pre_sb[:, fc, 128 * j:128 * (j + 1)],
                            identity=ident[:])
                hn = p_hn.tile([128, 4, 256], f16, name="hn", tag="hn")
                for j in range(4):
                    layernorm_affine(gT[:, j, :],
                                     None if ln0_trivial else g0t_s,
                                     None if ln0_trivial else b0t_s,
                                     None, 0.0, hn, j)
                nc.sync.dma_start(
                    out=hs[0][:].rearrange("(c p) f -> p c f", p=128)[:, 4 * w:4 * (w + 1), :],
                    in_=hn[:])
                write_ag_in(hn, w)

            # ---------------- 3 GINE layers
            for li in range(3):
                table = tables[li]
                nc.gpsimd.collective_compute(
                    "AllGather", Alu.bypass,
                    replica_groups=[list(range(NC_CORES))],
                    ins=[ag_in[:]], outs=[table[:]])

                m_tiles = {}

                def make_m(t, li=li, m_tiles=m_tiles, table=table):
                    hsrc_t = p_hsrc.tile([128, 256], f16, name=f"hsrc{t}", tag="hsrc")
                    nc.gpsimd.indirect_dma_start(
                        out=hsrc_t[:], out_offset=None, in_=table[:],
                        in_offset=bass.IndirectOffsetOnAxis(ap=gsrc_s[:, t:t + 1], axis=0))
                    esel_t = p_esel.tile([5, 128], f16, name=f"esel{t}", tag="esel")
                    nc.sync.dma_start(out=esel_t[:], in_=esel[:, 128 * t:128 * (t + 1)])
                    el = ps_el.tile([128, 256], f32, name="el", tag="el")
                    nc.tensor.matmul(out=el[:], lhsT=esel_t[:],
                                     rhs=Ttab_s[li][:], start=True, stop=False)
                    nc.tensor.matmul(out=el[:], lhsT=ident[:], rhs=hsrc_t[:],
                                     start=False, stop=True)
                    m_t = p_m.tile([128, 256], f16, name=f"m{t}", tag="m")
                    nc.scalar.activation(out=m_t[:], in_=el[:], func=Act.Relu)
                    m_tiles[t] = m_t
                    return m_t

                for w in range(NW):
                    hwin = p_hwin.tile([128, 4, 256], f16, name="hwin", tag="hwin")
                    nc.sync.dma_start(
                        out=hwin[:],
                        in_=hs[li][:].rearrange("(c p) f -> p c f", p=128)[:, 4 * w:4 * (w + 1), :])
                    xw_nm = p_xwn.tile([128, 4, 256], f16, name="xw_nm", tag="xwn")
                    for j in range(4):
                        sw = 4 * w + j
                        t_lo, t_hi = intervals[sw]
                        agg = ps_agg.tile([128, 256], f32, name="agg", tag="agg")
                        if t_lo >= t_hi:
                            nc.vector.memset(agg[:], 0.0)
                        for t in range(t_lo, t_hi):
                            m_t = m_tiles.get(t)
                            if m_t is None:
                                m_t = make_m(t)
                            S_t = p_S.tile([128, 128], f16, name="S_t", tag="S")
                            nc.vector.tensor_scalar(
                                out=S_t[:], in0=iota_s[:, 128 * sw:128 * (sw + 1)],
                                scalar1=dstloc_s[:, t:t + 1], scalar2=None,
                                op0=Alu.is_equal)
                            nc.tensor.matmul(out=agg[:], lhsT=S_t[:], rhs=m_t[:],
                                             start=(t == t_lo), stop=(t == t_hi - 1))
                        nc.vector.tensor_add(out=xw_nm[:, j, :], in0=hwin[:, j, :],
                                             in1=agg[:])
                    # transpose xw to feature-major
                    xwT = ps_t16.tile([128, 2, W], f16, name="xwT", tag="t16")
                    for j in range(4):
                        for fc in range(2):
                            nc.tensor.transpose(
                                out=xwT[:, fc, 128 * j:128 * (j + 1)],
                                in_=xw_nm[:, j, 128 * fc:128 * (fc + 1)],
                                identity=ident[:])
                    xw = p_xw.tile([128, 2, W], f16, name="xw", tag="xw")
                    nc.scalar.activation(out=xw[:], in_=xwT[:], func=Act.Copy)
                    # W1 -> relu -> W2 -> relu
                    y1p = [ps_mlp.tile([128, W], f32, name=f"y1p{fc}", tag="mlp")
                           for fc in range(2)]
                    for fc in range(2):
                        for ki in range(2):
                            nc.tensor.matmul(out=y1p[fc][:],
                                             lhsT=W1T_s[li][:, ki, 128 * fc:128 * (fc + 1)],
                                             rhs=xw[:, ki, :],
                                             start=(ki == 0), stop=(ki == 1))
                    y1 = p_act.tile([128, W], f16, name="y1", tag="act")
                    y1b = p_act.tile([128, W], f16, name="y1b", tag="act")
                    ys = [y1, y1b]
                    for fc in range(2):
                        nc.scalar.activation(out=ys[fc][:], in_=y1p[fc][:], func=Act.Relu,
                                             bias=b1_s[li][:, fc:fc + 1], scale=1.0)
                    y2p = [ps_mlp.tile([128, W], f32, name=f"y2p{fc}", tag="mlp")
                           for fc in range(2)]
                    for fc in range(2):
                        for ki in range(2):
                            nc.tensor.matmul(out=y2p[fc][:],
                                             lhsT=W2T_s[li][:, ki, 128 * fc:128 * (fc + 1)],
                                             rhs=ys[ki][:],
                                             start=(ki == 0), stop=(ki == 1))
                    g1 = p_act.tile([128, W], f16, name="g1", tag="act")
                    g2 = p_act.tile([128, W], f16, name="g2", tag="act")
                    gs = [g1, g2]
                    for fc in range(2):
                        nc.scalar.activation(out=gs[fc][:], in_=y2p[fc][:], func=Act.Relu,
                                             bias=b2_s[li][:, fc:fc + 1], scale=1.0)
                    gT = ps_t16.tile([128, 4, 256], f16, name="gT2", tag="t16")
                    for j in range(4):
                        for fc in range(2):
                            nc.tensor.transpose(
                                out=gT[:, j, 128 * fc:128 * (fc + 1)],
                                in_=gs[fc][:, 128 * j:128 * (j + 1)],
                                identity=ident[:])
                    hn = p_hn.tile([128, 4, 256], f16, name="hn2", tag="hn")
                    for j in range(4):
                        layernorm_affine(gT[:, j, :], gmt_s[li], btt_s[li],
                                         hwin, one_minus_r[li], hn, j)
                    nc.sync.dma_start(
                        out=hs[li + 1][:].rearrange("(c p) f -> p c f", p=128)[:, 4 * w:4 * (w + 1), :],
                        in_=hn[:])
                    if li < 2:
                        write_ag_in(hn, w)

            # ---------------- jumping knowledge head (2 windows per step)
            WJ = 2 * W   # 1024 rows per transpose load
            pending = []
            njs = (NPAD + WJ - 1) // WJ
            for wj in range(njs):
                nw_here = min(2, (NPAD - WJ * wj) // W)
                rows = slice(WJ * wj, WJ * wj + nw_here * W)
                hTc = [p_jk.tile([128, WJ], f16, name=f"hTc{c}", tag="jk")
                       for c in range(8)]
                for c in range(8):
                    i4, fc = c // 2, c % 2
                    nc.sync.dma_start(out=hTc[c][:, :nw_here * W],
                                      in_=hs[i4][rows, 128 * fc:128 * (fc + 1)],
                                      transpose=True)
                for half in range(nw_here):
                    w = 2 * wj + half
                    outp = [ps_mlp.tile([128, W], f32, name=f"op{oc}", tag="mlp")
                            for oc in range(2)]
                    for c in range(8):
                        for oc in range(2):
                            nc.tensor.matmul(out=outp[oc][:],
                                             lhsT=jkWT_s[:, c, 128 * oc:128 * (oc + 1)],
                                             rhs=hTc[c][:, W * half:W * (half + 1)],
                                             start=(c == 0), stop=(c == 7))
                    for oc in range(2):
                        ob = p_out.tile([128, W], f32, name="ob", tag="ob")
                        nc.vector.tensor_scalar(out=ob[:], in0=outp[oc][:],
                                                scalar1=jkb_s[:, oc:oc + 1],
                                                scalar2=None, op0=Alu.add)
                        pending.append((ob, oc, w))
                if len(pending) >= 8 or wj == njs - 1:
                    for ob, oc, w in pending:
                        nc.sync.dma_start(
                            out=outT[128 * oc:128 * (oc + 1), W * w:W * (w + 1)],
                            in_=ob[:])
                    pending = []

    _split_sync_waits(nc)
    return nc


# ---------------------------------------------------------------- entry point

_CACHE = {}


def kernel(atom_inputs, edge_index, edge_type, params):
    _install_patches()
    _install_ntff_hook()

    x = np.asarray(atom_inputs, dtype=np.float32)
    A1, A2 = fold_params(params)
    H = build_H124(x[:, :31])                     # [N,124] fp16
    H = np.concatenate([H, np.ones((H.shape[0], 1), np.float16)], 1)  # +const col
    benv = x[:, 31:].astype(np.float16)           # [N,48]
    gsrcs, dstlocs, esels, ntiles, epad, intervals = prep_graph(edge_index, edge_type)

    p = params
    one_minus_r = [1.0 - float(np.asarray(p[f'res{i}'])) for i in (1, 2, 3)]
    r_ = [float(np.asarray(p[f'res{i}'])) for i in (1, 2, 3)]

    common = {
        "iota": np.tile(np.arange(NPAD, dtype=np.float32), (128, 1)),
        "A1": A1.astype(np.float16),
        "A2": A2.astype(np.float16),
        "g0t": np.tile(_np(p['ln_in'][0]).astype(np.float16), (128, 1)),
        "b0t": np.tile(_np(p['ln_in'][1]).astype(np.float16), (128, 1)),
        "jkWT": _np(p['jk'][0]).T.astype(np.float16).reshape(8, 128, 256).transpose(1, 0, 2).copy(),
        "jkb": _np(p['jk'][1]).astype(np.float32).reshape(2, 128).T.copy(),
    }
    for i0, i in enumerate((1, 2, 3)):
        common[f"W1T{i0}"] = _np(p[f'gine{i}_W1'][0]).T.astype(np.float16).reshape(2, 128, 256).transpose(1, 0, 2).copy()
        common[f"W2T{i0}"] = _np(p[f'gine{i}_W2'][0]).T.astype(np.float16).reshape(2, 128, 256).transpose(1, 0, 2).copy()
        common[f"b1_{i0}"] = _np(p[f'gine{i}_W1'][1]).astype(np.float32).reshape(2, 128).T.copy()
        common[f"b2_{i0}"] = _np(p[f'gine{i}_W2'][1]).astype(np.float32).reshape(2, 128).T.copy()
        common[f"gmt{i0}"] = np.tile((r_[i0] * _np(p[f'ln{i}'][0])).astype(np.float16), (128, 1))
        common[f"btt{i0}"] = np.tile((r_[i0] * _np(p[f'ln{i}'][1])).astype(np.float16), (128, 1))
        common[f"Ttab{i0}"] = (_np(p['bond_emb']) @ _np(p[f'gine{i}_lin'][0]).T
                               + _np(p[f'gine{i}_lin'][1])).astype(np.float16)

    in_maps = []
    for k in range(NC_CORES):
        lo, hi = SLICE * k, SLICE * (k + 1)
        HTk = np.zeros((125, NPAD), np.float16)
        HTk[:, :SLICE] = H[lo:hi].T
        BTk = np.zeros((48, NPAD), np.float16)
        BTk[:, :SLICE] = benv[lo:hi].T
        m = dict(common)
        m.update({"HT": HTk, "BT": BTk, "gsrc": gsrcs[k],
                  "dstloc": dstlocs[k], "esel": esels[k]})
        in_maps.append(m)

    ln0_trivial = bool(np.all(_np(p['ln_in'][0]) == 1.0) and np.all(_np(p['ln_in'][1]) == 0.0))
    key = (ntiles, epad, tuple(intervals), tuple(one_minus_r), ln0_trivial)
    nc = _CACHE.get(key)
    if nc is None:
        nc = build_program(ntiles, epad, intervals, one_minus_r, ln0_trivial)
        _CACHE[key] = nc

    trace = bool(int(os.environ.get("GINE_TRACE", "0")))
    res = bass_utils.run_bass_kernel_spmd(nc, in_maps,
                                          core_ids=list(range(NC_CORES)),
                                          trace=trace)
    kernel.last_exec_time_ns = res.exec_time_ns
    out = np.concatenate(
        [res.results[k]["outT"].T[:SLICE] for k in range(NC_CORES)], axis=0)
    return out.astype(np.float32)


kernel.last_exec_time_ns = None


# revision 17
# speedup vs baseline: 1.3324x; 1.1263x over previous
"""Trainium2 Bass kernel for EquivariantThreeHopGINE (GNN message passing).

Strategy (8 NeuronCores, SPMD):
  - Nodes partitioned 12500/core (dst-sharding); edges assigned to the core
    owning their dst, sorted by dst, padded to a common tile count.
  - Atom embedding folded on host into a 124-wide multi-hot matmul
    (index preprocessing only; all float math stays on device).
  - Per GINE layer: gather h[src] from a replicated fp16 node table in DRAM
    (indirect DMA, 128 rows/call), messages m = relu(h_src + T[et]) via
    PE matmuls + ACT relu, scatter-add via one-hot matmuls into 512-node
    windows, then the W1/W2 MLP + LayerNorm + gated residual per window.
  - h slices are AllGathered (fp16) between layers to refresh the table.
  - Jumping-knowledge head: weight-stationary matmuls over DMA-transposed
    h0..h3 window slices.
"""
import os
import sys
import types
import numpy as np

for _p in ("/opt/trn_rl_repo", "/root/.axon_site/_ro/trn_rl_repo"):
    if os.path.isdir(_p) and _p not in sys.path:
        sys.path.insert(0, _p)

import concourse.bass as bass
import concourse.tile as tile
import concourse.mybir as mybir
from concourse import bass_utils
from concourse.masks import make_identity
import bass_rust

f32 = mybir.dt.float32
f16 = mybir.dt.float16
i32 = mybir.dt.int32
Alu = mybir.AluOpType
Act = mybir.ActivationFunctionType

N = 100000
E = 300000
HID = 256
NC_CORES = 8
SLICE = N // NC_CORES          # 12500
W = 512                        # dst-window width (nodes)
NW = (SLICE + W - 1) // W      # 25
NPAD = NW * W                  # 12800
NSW = NPAD // 128              # 100 scatter sub-windows

ELEMENTS = [5, 6, 7, 8, 14, 15, 16]
ELEMENT_LUT = np.zeros(17, dtype=np.int64)
for _i, _z in enumerate(ELEMENTS):
    ELEMENT_LUT[_z] = _i
RING_VALS = np.array([0, 3, 4, 5, 6, 7, 8], dtype=np.int64)

# ---------------------------------------------------------------- compat shims


def _install_patches():
    """Split multi-sem-wait instructions: the public neuronxcc walrus codegen
    supports a single sync wait per instruction."""
    SC = bass_rust.ScopedClock

    def patched_drain(self, tick_clock, wait_clock):
        nc = self.nc
        drain_inst = nc.sync.drain()
        wait_clock.add_sem_waits(drain_inst.ins, SC({None: tick_clock.global_clock}))
        si = drain_inst.ins.sync_info
        waits = list(si.on_wait or [])
        if len(waits) > 1:
            si.on_wait = waits[:1]
            for w in waits[1:]:
                n = nc.sync.nop(nofuse=True)
                if n.ins.sync_info is None:
                    n.ins.sync_info = mybir.SyncInfo(on_wait=[w], on_update=[])
                else:
                    n.ins.sync_info.on_wait = [w]
        nc.all_engine_barrier()
        popped = nc._tile_sem_poison_stack.pop()
        assert popped is self._sem_poison
        nc.clear_and_free_semaphores(list(self.sems.allocated().values()))
        nc.all_engine_barrier()

    tile.TileContext._drain_and_barrier = patched_drain


_WAIT_UID = [0]


def _split_sync_waits(nc, maxw=1):
    for fn in nc.m.functions:
        for blk in fn.blocks:
            newlist = []
            for inst in blk.instructions:
                si = inst.sync_info
                if si is not None and si.on_wait and len(si.on_wait) > maxw:
                    waits = list(si.on_wait)
                    si.on_wait = waits[:maxw]
                    extra = waits[maxw:]
                    for i in range(0, len(extra), maxw):
                        _WAIT_UID[0] += 1
                        n = mybir.InstNoOp(name=f"waitnop_{_WAIT_UID[0]}", ins=[], outs=[])
                        n.engine = inst.engine
                        n.sync_info = mybir.SyncInfo(on_wait=list(extra[i:i + maxw]), on_update=[])
                        newlist.append(n)
                newlist.append(inst)
            blk.instructions[:] = newlist


def _install_ntff_hook():
    try:
        import antenv.axon_hooks  # noqa: F401
        return True
    except ImportError:
        pass
    try:
        if '/root/.axon_site' not in sys.path:
            sys.path.insert(0, '/root/.axon_site')
        from trn_agent_boot.trn_boot import _ntff_profile_via_ctypes
        hook = _ntff_profile_via_ctypes('/opt/axon/libaxon_pjrt.so')
        if hook is None:
            return False
        mod = types.ModuleType("antenv.axon_hooks")
        mod.get_axon_ntff_profile_hook = lambda: hook
        mod.set_axon_ntff_profile_hook = lambda h: None
        sys.modules["antenv.axon_hooks"] = mod
        import antenv
        antenv.axon_hooks = mod
        return True
    except Exception:
        return False


# ---------------------------------------------------------------- host folding

def _np(x):
    return np.asarray(x, dtype=np.float32)


def fold_params(p):
    """Fold the atom-embedding pipeline into A1[124,256], A2[48,256], c0[256]."""
    P, p_b = _np(p['disc_proj'][0]), _np(p['disc_proj'][1])      # [48,52],[48]
    L, l_b = _np(p['linear_0'][0]), _np(p['linear_0'][1])        # [256,64],[256]
    B, b_be = _np(p['bond_env_proj'][0]), _np(p['bond_env_proj'][1])  # [16,48],[16]
    R, r_b = _np(p['func_reduce'][0]), _np(p['func_reduce'][1])  # [4,36],[4]
    L48 = L[:, :48]   # [256,48]
    L16 = L[:, 48:]   # [256,16]
    G = L48 @ P       # [256,52]

    # table spec: (52-offset, dim, rows[n,dim])
    specs = []
    specs.append((0, 4, _np(p['element'])))      # 7
    specs.append((4, 4, _np(p['degree'])))       # 7
    specs.append((8, 4, _np(p['ring'])))         # 2
    specs.append((12, 4, _np(p['charge'])))      # 8
    specs.append((16, 4, _np(p['aromatic'])))    # 2
    specs.append((20, 4, _np(p['hybrid'])))      # 6
    specs.append((24, 4, _np(p['hydrogen'])))    # 5
    func = _np(p['func'])                        # [18,2,2]
    for j in range(18):
        rows = func[j] @ R[:, 2 * j:2 * j + 2].T   # [2,4] contribution to flags4
        specs.append((28, 4, rows))
    specs.append((32, 2, _np(p['h_don'])))       # 2
    specs.append((34, 2, _np(p['h_acc'])))       # 2
    specs.append((36, 4, _np(p['ringsize'])))    # 7
    specs.append((40, 4, _np(p['aroma_num'])))   # 5
    specs.append((44, 4, _np(p['fused_if'])))    # 8
    specs.append((48, 4, _np(p['het27'])))       # 27

    rows = []
    for off, dim, tab in specs:
        for k in range(tab.shape[0]):
            v52 = np.zeros(52, np.float32)
            v52[off:off + dim] = tab[k]
            rows.append(v52 @ G.T)               # [256]
    A1 = np.stack(rows, 0)                       # [124, 256]
    assert A1.shape[0] == 124

    const52 = np.zeros(52, np.float32)
    const52[28:32] = r_b
    c0 = const52 @ G.T + p_b @ L48.T + b_be @ L16.T + l_b   # [256]
    A1 = np.concatenate([A1, c0[None, :]], 0)                # [125, 256]
    A2 = B.T @ L16.T                                         # [48, 256]
    return A1, A2


def build_H124(xd):
    """Multi-hot index matrix from discrete atom columns (pure indexing)."""
    n = xd.shape[0]
    xi = xd.astype(np.int64)

    def clip(c, hi):
        return np.clip(xi[:, c], 0, hi)

    idxs = []
    z = xi[:, 0]
    z_safe = np.where((z >= 0) & (z <= 16), z, 0)
    idxs.append(np.clip(ELEMENT_LUT[z_safe], 0, 6))          # element 7
    idxs.append(clip(1, 6))                                   # degree 7
    idxs.append(np.clip(xi[:, 5] + 1, 0, 1))                  # ring 2
    idxs.append(clip(2, 7))                                   # charge 8
    idxs.append(clip(4, 1))                                   # aromatic 2
    idxs.append(clip(3, 5))                                   # hybrid 6
    idxs.append(clip(6, 4))                                   # hydrogen 5
    for j in range(18):
        idxs.append(clip(7 + j, 1))                           # flags 2 each
    idxs.append(clip(25, 1))                                  # h_don 2
    idxs.append(clip(26, 1))                                  # h_acc 2
    raw27 = xi[:, 27]
    match = raw27[:, None] == RING_VALS[None, :]
    mapped27 = np.where(match.any(1), match.argmax(1), 6)
    idxs.append(mapped27)                                     # ringsize 7
    idxs.append(clip(28, 4))                                  # aroma_num 5
    idxs.append(clip(29, 7))                                  # fused_if 8
    idxs.append(clip(30, 26))                                 # het27 27

    sizes = [7, 7, 2, 8, 2, 6, 5] + [2] * 18 + [2, 2, 7, 5, 8, 27]
    H = np.zeros((n, 124), np.float16)
    base = 0
    for sz, ix in zip(sizes, idxs):
        H[np.arange(n), base + ix] = 1.0
        base += sz
    assert base == 124
    return H


def prep_graph(edge_index, edge_type):
    """Per-core dst-sharded, dst-sorted edge arrays + shared tiling structure."""
    src = np.asarray(edge_index[0]).astype(np.int64)
    dst = np.asarray(edge_index[1]).astype(np.int64)
    et = np.asarray(edge_type).astype(np.int64)

    cores = []
    for k in range(NC_CORES):
        lo, hi = SLICE * k, SLICE * (k + 1)
        sel = (dst >= lo) & (dst < hi)
        d = dst[sel] - lo
        order = np.argsort(d, kind='stable')
        cores.append((src[sel][order], d[order], np.clip(et[sel][order], 0, 4)))

    emax = max(len(c[0]) for c in cores)
    ntiles = (emax + 127) // 128
    epad = ntiles * 128

    gsrcs, dstlocs, esels = [], [], []
    intervals = [[ntiles, 0] for _ in range(NSW)]
    for k in range(NC_CORES):
        s, d, t = cores[k]
        ne = len(s)
        gs = np.zeros(epad, np.int32)
        gs[:ne] = s
        dl = np.full(epad, -100000.0, np.float32)
        dl[:ne] = d
        es = np.zeros((5, epad), np.float16)
        es[t, np.arange(ne)] = 1.0
        gsrcs.append(gs.reshape(ntiles, 128).T.copy())        # [128, ntiles]
        dstlocs.append(dl.reshape(ntiles, 128).T.copy())      # [128, ntiles]
        esels.append(es)
        wofe = d // 128                                       # sub-window of edge
    for k in range(NC_CORES):
        s, d, t = cores[k]
        wofe = d // 128
        for w in range(NSW):
            pos = np.nonzero(wofe == w)[0]
            if len(pos):
                intervals[w][0] = min(intervals[w][0], int(pos[0]) // 128)
                intervals[w][1] = max(intervals[w][1], int(pos[-1]) // 128 + 1)
    for w in range(NSW):
        if intervals[w][0] >= intervals[w][1]:
            intervals[w] = [0, 0]
    sw_lo = [NSW] * ntiles
    sw_hi = [0] * ntiles
    for w in range(NSW):
        for t in range(intervals[w][0], intervals[w][1]):
            sw_lo[t] = min(sw_lo[t], w)
            sw_hi[t] = max(sw_hi[t], w + 1)
    sww = max((hi - lo) for lo, hi in zip(sw_lo, sw_hi) if hi > 0)
    return (gsrcs, dstlocs, esels, ntiles, epad,
            [tuple(x) for x in intervals], sw_lo, sww)


# ---------------------------------------------------------------- bass program

def build_program(ntiles, epad, intervals, sw_lo, sww, one_minus_r, ln0_trivial):
    nc = bass.Bass("TRN2", target_bir_lowering=False, debug=False,
                   num_devices=NC_CORES)

    def din(name, shape, dt):
        return nc.dram_tensor(name, shape, dt, kind="ExternalInput").ap()

    HT = din("HT", [125, NPAD], f16)
    BT = din("BT", [48, NPAD], f16)
    gsrc = din("gsrc", [128, ntiles], i32)
    dstloc = din("dstloc", [128, ntiles], f32)
    esel = din("esel", [5, epad], f16)
    iota = din("iota", [128, 384], f32)
    A1 = din("A1", [125, 256], f16)
    A2 = din("A2", [48, 256], f16)
    g0t = din("g0t", [128, 4, 256], f16)
    b0t = din("b0t", [128, 4, 256], f16)
    W1T = [din(f"W1T{i}", [128, 2, 256], f16) for i in range(3)]
    W2T = [din(f"W2T{i}", [128, 2, 256], f16) for i in range(3)]
    b1 = [din(f"b1_{i}", [128, 2], f32) for i in range(3)]
    b2 = [din(f"b2_{i}", [128, 2], f32) for i in range(3)]
    gmt = [din(f"gmt{i}", [128, 4, 256], f16) for i in range(3)]
    btt = [din(f"btt{i}", [128, 4, 256], f16) for i in range(3)]
    Ttab = [din(f"Ttab{i}", [5, 256], f16) for i in range(3)]
    jkWT = din("jkWT", [128, 8, 256], f16)
    jkb = din("jkb", [128, 2], f32)
    outT = nc.dram_tensor("outT", [256, NPAD], f32, kind="ExternalOutput").ap()

    from contextlib import ExitStack
    with tile.TileContext(nc, num_cores=NC_CORES) as tc:
        with ExitStack() as ctx:
            ent = ctx.enter_context
            dram = ent(tc.tile_pool(name="dram", bufs=1, space="DRAM"))
            cst = ent(tc.tile_pool(name="consts", bufs=1))
            p_hsrc = ent(tc.tile_pool(name="p_hsrc", bufs=24))
            p_esel = ent(tc.tile_pool(name="p_esel", bufs=24))
            p_hb = ent(tc.tile_pool(name="p_hb", bufs=3))
            p_m = ent(tc.tile_pool(name="p_m", bufs=24))
            p_S = ent(tc.tile_pool(name="p_S", bufs=24))
            p_hwin = ent(tc.tile_pool(name="p_hwin", bufs=3))
            p_xw = ent(tc.tile_pool(name="p_xw", bufs=3))
            p_xwn = ent(tc.tile_pool(name="p_xwn", bufs=2))
            p_act = ent(tc.tile_pool(name="p_act", bufs=8))
            p_ln = ent(tc.tile_pool(name="p_ln", bufs=2))
            p_lns = ent(tc.tile_pool(name="p_lns", bufs=8))
            p_hn = ent(tc.tile_pool(name="p_hn", bufs=3))
            p_jk = ent(tc.tile_pool(name="p_jk", bufs=8))
            p_out = ent(tc.tile_pool(name="p_out", bufs=8))
            ps_agg = ent(tc.tile_pool(name="ps_agg", bufs=2, space="PSUM"))
            ps_mlp = ent(tc.tile_pool(name="ps_mlp", bufs=2, space="PSUM"))
            ps_t16 = ent(tc.tile_pool(name="ps_t16", bufs=2, space="PSUM"))
            ps_el = ent(tc.tile_pool(name="ps_el", bufs=2, space="PSUM"))

            hs = [dram.tile([NPAD, 256], f16, name=f"hs{i}") for i in range(4)]
            ag_in = dram.tile([SLICE, 256], f16, name="ag_in")
            tables = [dram.tile([N, 256], f16, addr_space="Shared", name=f"table{i}")
                      for i in range(3)]

            def load(ap_in, shape, dt, name):
                t = cst.tile(shape, dt, name=name)
                nc.sync.dma_start(out=t[:], in_=ap_in)
                return t

            gsrc_s = load(gsrc, [128, ntiles], i32, "gsrc_s")
            dstloc_s = load(dstloc, [128, ntiles], f32, "dstloc_s")
            iota_s = load(iota, [128, 384], f32, "iota_s")
            A1_s = load(A1, [125, 256], f16, "A1_s")
            A2_s = load(A2, [48, 256], f16, "A2_s")
            g0t_s = load(g0t, [128, 4, 256], f16, "g0t_s")
            b0t_s = load(b0t, [128, 4, 256], f16, "b0t_s")
            W1T_s = [load(W1T[i], [128, 2, 256], f16, f"W1T_s{i}") for i in range(3)]
            W2T_s = [load(W2T[i], [128, 2, 256], f16, f"W2T_s{i}") for i in range(3)]
            b1_s = [load(b1[i], [128, 2], f32, f"b1_s{i}") for i in range(3)]
            b2_s = [load(b2[i], [128, 2], f32, f"b2_s{i}") for i in range(3)]
            gmt_s = [load(gmt[i], [128, 4, 256], f16, f"gmt_s{i}") for i in range(3)]
            btt_s = [load(btt[i], [128, 4, 256], f16, f"btt_s{i}") for i in range(3)]
            Ttab_s = [load(Ttab[i], [5, 256], f16, f"Ttab_s{i}") for i in range(3)]
            jkWT_s = load(jkWT, [128, 8, 256], f16, "jkWT_s")
            jkb_s = load(jkb, [128, 2], f32, "jkb_s")

            ident = cst.tile([128, 128], f16, name="ident")
            make_identity(nc, ident[:])
            eps_s = cst.tile([128, 1], f32, name="eps_s")
            nc.vector.memset(eps_s[:], 1e-5)

            def write_ag_in(hn_t, w):
                base = W * w
                if base + W <= SLICE:
                    nc.sync.dma_start(
                        out=ag_in[base:base + W, :].rearrange("(c p) f -> p c f", p=128),
                        in_=hn_t[:])
                else:
                    rem = SLICE - base
                    full = rem // 128
                    for j in range(full):
                        nc.sync.dma_start(out=ag_in[base + 128 * j:base + 128 * (j + 1), :],
                                          in_=hn_t[:, j, :])
                    part = rem - full * 128
                    if part:
                        nc.sync.dma_start(out=ag_in[base + full * 128:SLICE, :],
                                          in_=hn_t[:part, full, :])

            def layernorm_affine4(g_sb4, gamt4, bett4, hwin_s, omr, hn_t):
                """Batched node-major LN over [128, 4, 256] SBUF input."""
                mv4 = p_lns.tile([128, 4, 2], f32, name="mv4", tag="mv4")
                for j in range(4):
                    stats = p_lns.tile([128, 6], f32, name="stats", tag="stats")
                    nc.vector.bn_stats(out=stats[:], in_=g_sb4[:, j, :])
                    nc.vector.bn_aggr(out=mv4[:, j, :], in_=stats[:])
                std4 = p_lns.tile([128, 4, 1], f32, name="std4", tag="std4")
                nc.scalar.activation(out=std4[:], in_=mv4[:, :, 1:2], func=Act.Sqrt,
                                     bias=eps_s[:, :1], scale=1.0)
                rstd4 = p_lns.tile([128, 4, 1], f32, name="rstd4", tag="rstd4")
                nc.vector.reciprocal(out=rstd4[:], in_=std4[:])
                cen = p_ln.tile([128, 4, 256], f16, name="cen", tag="cen")
                nc.vector.tensor_tensor(out=cen[:], in0=g_sb4[:],
                                        in1=mv4[:, :, 0:1].to_broadcast([128, 4, 256]),
                                        op=Alu.subtract)
                if gamt4 is None and hwin_s is None:
                    nc.vector.tensor_tensor(out=hn_t[:], in0=cen[:],
                                            in1=rstd4[:].to_broadcast([128, 4, 256]),
                                            op=Alu.mult)
                    return
                u = p_ln.tile([128, 4, 256], f16, name="u", tag="u")
                nc.vector.tensor_tensor(out=u[:], in0=cen[:],
                                        in1=rstd4[:].to_broadcast([128, 4, 256]),
                                        op=Alu.mult)
                v = p_ln.tile([128, 4, 256], f16, name="v", tag="v")
                nc.vector.tensor_mul(out=v[:], in0=u[:], in1=gamt4[:])
                if hwin_s is None:
                    nc.vector.tensor_add(out=hn_t[:], in0=v[:], in1=bett4[:])
                    return
                v2 = p_ln.tile([128, 4, 256], f16, name="v2", tag="v2")
                nc.vector.tensor_add(out=v2[:], in0=v[:], in1=bett4[:])
                hres = p_ln.tile([128, 4, 256], f16, name="hres", tag="hres")
                nc.scalar.mul(out=hres[:], in_=hwin_s[:], mul=float(omr))
                nc.vector.tensor_add(out=hn_t[:], in0=v2[:], in1=hres[:])

            # ---------------- phase 0: atom embed -> h0
            for w in range(NW):
                ncol = slice(W * w, W * (w + 1))
                ht_t = p_hb.tile([125, W], f16, name="ht_t", tag="ht")
                nc.sync.dma_start(out=ht_t[:], in_=HT[:, ncol])
                bt_t = p_hb.tile([48, W], f16, name="bt_t", tag="bt")
                nc.sync.dma_start(out=bt_t[:], in_=BT[:, ncol])
                pre = [ps_mlp.tile([128, W], f32, name=f"pre{fc}", tag="mlp")
                       for fc in range(2)]
                for fc in range(2):
                    nc.tensor.matmul(out=pre[fc][:], lhsT=A1_s[:, 128 * fc:128 * (fc + 1)],
                                     rhs=ht_t[:], start=True, stop=False)
                    nc.tensor.matmul(out=pre[fc][:], lhsT=A2_s[:, 128 * fc:128 * (fc + 1)],
                                     rhs=bt_t[:], start=False, stop=True)
                pre_sb = p_xw.tile([128, 2, W], f16, name="pre_sb", tag="xw")
                for fc in range(2):
                    nc.scalar.activation(out=pre_sb[:, fc, :], in_=pre[fc][:],
                                         func=Act.Copy)
                gT = ps_t16.tile([128, 4, 256], f16, name="gT", tag="t16")
                for j in range(4):
                    for fc in range(2):
                        nc.tensor.transpose(
                            out=gT[:, j, 128 * fc:128 * (fc + 1)],
                            in_=pre_sb[:, fc, 128 * j:128 * (j + 1)],
                            identity=ident[:])
                g_sb = p_ln.tile([128, 4, 256], f16, name="g_sb", tag="gsb")
                nc.scalar.activation(out=g_sb[:], in_=gT[:], func=Act.Copy)
                hn = p_hn.tile([128, 4, 256], f16, name="hn", tag="hn")
                layernorm_affine4(g_sb,
                                  None if ln0_trivial else g0t_s,
                                  None if ln0_trivial else b0t_s,
                                  None, 0.0, hn)
                nc.sync.dma_start(
                    out=hs[0][:].rearrange("(c p) f -> p c f", p=128)[:, 4 * w:4 * (w + 1), :],
                    in_=hn[:])
                write_ag_in(hn, w)

            # ---------------- 3 GINE layers
            for li in range(3):
                table = tables[li]
                nc.gpsimd.collective_compute(
                    "AllGather", Alu.bypass,
                    replica_groups=[list(range(NC_CORES))],
                    ins=[ag_in[:]], outs=[table[:]])

                m_tiles = {}

                def make_m(t, li=li, m_tiles=m_tiles, table=table):
                    hsrc_t = p_hsrc.tile([128, 256], f16, name=f"hsrc{t}", tag="hsrc")
                    nc.gpsimd.indirect_dma_start(
                        out=hsrc_t[:], out_offset=None, in_=table[:],
                        in_offset=bass.IndirectOffsetOnAxis(ap=gsrc_s[:, t:t + 1], axis=0))
                    esel_t = p_esel.tile([5, 128], f16, name=f"esel{t}", tag="esel")
                    nc.sync.dma_start(out=esel_t[:], in_=esel[:, 128 * t:128 * (t + 1)])
                    el = ps_el.tile([128, 256], f32, name="el", tag="el")
                    nc.tensor.matmul(out=el[:], lhsT=esel_t[:],
                                     rhs=Ttab_s[li][:], start=True, stop=False)
                    nc.tensor.matmul(out=el[:], lhsT=ident[:], rhs=hsrc_t[:],
                                     start=False, stop=True)
                    m_t = p_m.tile([128, 256], f16, name=f"m{t}", tag="m")
                    nc.scalar.activation(out=m_t[:], in_=el[:], func=Act.Relu)
                    lo = sw_lo[t]
                    sh = p_lns.tile([128, 1], f32, name="sh", tag="sh")
                    nc.vector.tensor_scalar(out=sh[:], in0=dstloc_s[:, t:t + 1],
                                            scalar1=float(128 * lo), scalar2=None,
                                            op0=Alu.subtract)
                    S_t = p_S.tile([128, 128 * sww], f16, name=f"S{t}", tag="S")
                    nc.vector.tensor_scalar(
                        out=S_t[:], in0=iota_s[:, :128 * sww],
                        scalar1=sh[:, 0:1], scalar2=None,
                        op0=Alu.is_equal)
                    m_tiles[t] = (m_t, S_t, lo)
                    return m_tiles[t]

                for w in range(NW):
                    hwin = p_hwin.tile([128, 4, 256], f16, name="hwin", tag="hwin")
                    nc.sync.dma_start(
                        out=hwin[:],
                        in_=hs[li][:].rearrange("(c p) f -> p c f", p=128)[:, 4 * w:4 * (w + 1), :])
                    xw_nm = p_xwn.tile([128, 4, 256], f16, name="xw_nm", tag="xwn")
                    for j in range(4):
                        sw = 4 * w + j
                        t_lo, t_hi = intervals[sw]
                        agg = ps_agg.tile([128, 256], f32, name="agg", tag="agg")
                        if t_lo >= t_hi:
                            nc.vector.memset(agg[:], 0.0)
                        for t in range(t_lo, t_hi):
                            ent_t = m_tiles.get(t)
                            if ent_t is None:
                                ent_t = make_m(t)
                            m_t, S_t, lo = ent_t
                            off = 128 * (sw - lo)
                            nc.tensor.matmul(out=agg[:], lhsT=S_t[:, off:off + 128],
                                             rhs=m_t[:],
                                             start=(t == t_lo), stop=(t == t_hi - 1))
                        nc.vector.tensor_add(out=xw_nm[:, j, :], in0=hwin[:, j, :],
                                             in1=agg[:])
                    # transpose xw to feature-major
                    xwT = ps_t16.tile([128, 2, W], f16, name="xwT", tag="t16")
                    for j in range(4):
                        for fc in range(2):
                            nc.tensor.transpose(
                                out=xwT[:, fc, 128 * j:128 * (j + 1)],
                                in_=xw_nm[:, j, 128 * fc:128 * (fc + 1)],
                                identity=ident[:])
                    xw = p_xw.tile([128, 2, W], f16, name="xw", tag="xw")
                    nc.scalar.activation(out=xw[:], in_=xwT[:], func=Act.Copy)
                    # W1 -> relu -> W2 -> relu
                    y1p = [ps_mlp.tile([128, W], f32, name=f"y1p{fc}", tag="mlp")
                           for fc in range(2)]
                    for fc in range(2):
                        for ki in range(2):
                            nc.tensor.matmul(out=y1p[fc][:],
                                             lhsT=W1T_s[li][:, ki, 128 * fc:128 * (fc + 1)],
                                             rhs=xw[:, ki, :],
                                             start=(ki == 0), stop=(ki == 1))
                    y1 = p_act.tile([128, W], f16, name="y1", tag="act")
                    y1b = p_act.tile([128, W], f16, name="y1b", tag="act")
                    ys = [y1, y1b]
                    for fc in range(2):
                        nc.scalar.activation(out=ys[fc][:], in_=y1p[fc][:], func=Act.Relu,
                                             bias=b1_s[li][:, fc:fc + 1], scale=1.0)
                    y2p = [ps_mlp.tile([128, W], f32, name=f"y2p{fc}", tag="mlp")
                           for fc in range(2)]
                    for fc in range(2):
                        for ki in range(2):
                            nc.tensor.matmul(out=y2p[fc][:],
                                             lhsT=W2T_s[li][:, ki, 128 * fc:128 * (fc + 1)],
                                             rhs=ys[ki][:],
                                             start=(ki == 0), stop=(ki == 1))
                    g1 = p_act.tile([128, W], f16, name="g1", tag="act")
                    g2 = p_act.tile([128, W], f16, name="g2", tag="act")
                    gs = [g1, g2]
                    for fc in range(2):
                        nc.scalar.activation(out=gs[fc][:], in_=y2p[fc][:], func=Act.Relu,
                                             bias=b2_s[li][:, fc:fc + 1], scale=1.0)
                    gT = ps_t16.tile([128, 4, 256], f16, name="gT2", tag="t16")
                    for j in range(4):
                        for fc in range(2):
                            nc.tensor.transpose(
                                out=gT[:, j, 128 * fc:128 * (fc + 1)],
                                in_=gs[fc][:, 128 * j:128 * (j + 1)],
                                identity=ident[:])
                    g_sb = p_ln.tile([128, 4, 256], f16, name="g_sb2", tag="gsb")
                    nc.scalar.activation(out=g_sb[:], in_=gT[:], func=Act.Copy)
                    hn = p_hn.tile([128, 4, 256], f16, name="hn2", tag="hn")
                    layernorm_affine4(g_sb, gmt_s[li], btt_s[li],
                                      hwin, one_minus_r[li], hn)
                    nc.sync.dma_start(
                        out=hs[li + 1][:].rearrange("(c p) f -> p c f", p=128)[:, 4 * w:4 * (w + 1), :],
                        in_=hn[:])
                    if li < 2:
                        write_ag_in(hn, w)

            # ---------------- jumping knowledge head (2 windows per step)
            WJ = 2 * W   # 1024 rows per transpose load
            pending = []
            njs = (NPAD + WJ - 1) // WJ
            for wj in range(njs):
                nw_here = min(2, (NPAD - WJ * wj) // W)
                rows = slice(WJ * wj, WJ * wj + nw_here * W)
                hTc = [p_jk.tile([128, WJ], f16, name=f"hTc{c}", tag="jk")
                       for c in range(8)]
                for c in range(8):
                    i4, fc = c // 2, c % 2
                    nc.sync.dma_start(out=hTc[c][:, :nw_here * W],
                                      in_=hs[i4][rows, 128 * fc:128 * (fc + 1)],
                                      transpose=True)
                for half in range(nw_here):
                    w = 2 * wj + half
                    outp = [ps_mlp.tile([128, W], f32, name=f"op{oc}", tag="mlp")
                            for oc in range(2)]
                    for c in range(8):
                        for oc in range(2):
                            nc.tensor.matmul(out=outp[oc][:],
                                             lhsT=jkWT_s[:, c, 128 * oc:128 * (oc + 1)],
                                             rhs=hTc[c][:, W * half:W * (half + 1)],
                                             start=(c == 0), stop=(c == 7))
                    for oc in range(2):
                        ob = p_out.tile([128, W], f32, name="ob", tag="ob")
                        nc.vector.tensor_scalar(out=ob[:], in0=outp[oc][:],
                                                scalar1=jkb_s[:, oc:oc + 1],
                                                scalar2=None, op0=Alu.add)
                        pending.append((ob, oc, w))
                if len(pending) >= 8 or wj == njs - 1:
                    for ob, oc, w in pending:
                        nc.sync.dma_start(
                            out=outT[128 * oc:128 * (oc + 1), W * w:W * (w + 1)],
                            in_=ob[:])
                    pending = []

    _split_sync_waits(nc)
    return nc


# ---------------------------------------------------------------- entry point

_CACHE = {}


def kernel(atom_inputs, edge_index, edge_type, params):
    _install_patches()
    _install_ntff_hook()

    x = np.asarray(atom_inputs, dtype=np.float32)
    A1, A2 = fold_params(params)
    H = build_H124(x[:, :31])                     # [N,124] fp16
    H = np.concatenate([H, np.ones((H.shape[0], 1), np.float16)], 1)  # +const col
    benv = x[:, 31:].astype(np.float16)           # [N,48]
    (gsrcs, dstlocs, esels, ntiles, epad, intervals,
     sw_lo, sww) = prep_graph(edge_index, edge_type)

    p = params
    one_minus_r = [1.0 - float(np.asarray(p[f'res{i}'])) for i in (1, 2, 3)]
    r_ = [float(np.asarray(p[f'res{i}'])) for i in (1, 2, 3)]

    common = {
        "iota": np.tile(np.arange(384, dtype=np.float32), (128, 1)),
        "A1": A1.astype(np.float16),
        "A2": A2.astype(np.float16),
        "g0t": np.tile(_np(p['ln_in'][0]).astype(np.float16), (128, 4, 1)),
        "b0t": np.tile(_np(p['ln_in'][1]).astype(np.float16), (128, 4, 1)),
        "jkWT": _np(p['jk'][0]).T.astype(np.float16).reshape(8, 128, 256).transpose(1, 0, 2).copy(),
        "jkb": _np(p['jk'][1]).astype(np.float32).reshape(2, 128).T.copy(),
    }
    for i0, i in enumerate((1, 2, 3)):
        common[f"W1T{i0}"] = _np(p[f'gine{i}_W1'][0]).T.astype(np.float16).reshape(2, 128, 256).transpose(1, 0, 2).copy()
        common[f"W2T{i0}"] = _np(p[f'gine{i}_W2'][0]).T.astype(np.float16).reshape(2, 128, 256).transpose(1, 0, 2).copy()
        common[f"b1_{i0}"] = _np(p[f'gine{i}_W1'][1]).astype(np.float32).reshape(2, 128).T.copy()
        common[f"b2_{i0}"] = _np(p[f'gine{i}_W2'][1]).astype(np.float32).reshape(2, 128).T.copy()
        common[f"gmt{i0}"] = np.tile((r_[i0] * _np(p[f'ln{i}'][0])).astype(np.float16), (128, 4, 1))
        common[f"btt{i0}"] = np.tile((r_[i0] * _np(p[f'ln{i}'][1])).astype(np.float16), (128, 4, 1))
        common[f"Ttab{i0}"] = (_np(p['bond_emb']) @ _np(p[f'gine{i}_lin'][0]).T
                               + _np(p[f'gine{i}_lin'][1])).astype(np.float16)

    in_maps = []
    for k in range(NC_CORES):
        lo, hi = SLICE * k, SLICE * (k + 1)
        HTk = np.zeros((125, NPAD), np.float16)
        HTk[:, :SLICE] = H[lo:hi].T
        BTk = np.zeros((48, NPAD), np.float16)
        BTk[:, :SLICE] = benv[lo:hi].T
        m = dict(common)
        m.update({"HT": HTk, "BT": BTk, "gsrc": gsrcs[k],
                  "dstloc": dstlocs[k], "esel": esels[k]})
        in_maps.append(m)

    ln0_trivial = bool(np.all(_np(p['ln_in'][0]) == 1.0) and np.all(_np(p['ln_in'][1]) == 0.0))
    key = (ntiles, epad, tuple(intervals), tuple(sw_lo), sww,
           tuple(one_minus_r), ln0_trivial)
    nc = _CACHE.get(key)
    if nc is None:
        nc = build_program(ntiles, epad, intervals, sw_lo, sww,
                           one_minus_r, ln0_trivial)
        _CACHE[key] = nc

    trace = bool(int(os.environ.get("GINE_TRACE", "0")))
    res = bass_utils.run_bass_kernel_spmd(nc, in_maps,
                                          core_ids=list(range(NC_CORES)),
                                          trace=trace)
    kernel.last_exec_time_ns = res.exec_time_ns
    out = np.concatenate(
        [res.results[k]["outT"].T[:SLICE] for k in range(NC_CORES)], axis=0)
    return out.astype(np.float32)


kernel.last_exec_time_ns = None
